# revision 8
# baseline (speedup 1.0000x reference)
"""CRF mean-field (nn_CRF) Trainium2 kernel, SPMD over 8 NeuronCores. v3.

Math: 5 iterations of
    p   = softmax(q, axis=classes)
    q   = unaries - compat @ (sw @ (p @ K_sp) + bw @ (p @ K_bl))

Design (v3, rewritten from v2 to shorten the per-iteration critical path):

  * The CxC mixing commutes with the N-axis filtering, so the mixing is
    PRE-applied to the local p shard before the AllGather:
        pb = (-(compat@bw)/(2*gamma)) @ p   (feeds the bilateral slab matmul)
        ps = (-(compat@sw)) @ p             (feeds the spatial poly path)
    via two tiny PE matmuls (transpose against identity, then a
    block-diagonal [40,80] mix matrix). Only pb is AllGathered; the
    spatial path ships as a per-core partial W_r = Psi_loc @ ps_loc^T
    (f32, byte-packed into the same AG buffer) and the 16 partials are
    tree-summed post-AG. This removes the per-iteration 64-chunk wtp
    accumulation and all post-filter mixing matmuls.

  * Main bilateral matmul: pb8 [128,32] fp8 stationary (zero-padded
    cols 10-31 so every PSUM row is defined) x slab [128,512] fp8
    moving, 4 PE column groups. The 4 band partials + the spatial osp
    (accumulated into band 0) are summed by ONE [128,128]x[128,10]
    sel-matmul per j-chunk after a single PSUM->SBUF bf16 copy --
    replacing v2's serial 2.7us DVE combine chain per half.

  * AllGathers (2/iter, 10KB) overlap compute: the k-loop is ordered
    AG0-chunks then AG1-chunks, and the j-half-1 mains + post run under
    AG0's flight.

  * Slab build unchanged numerically (exp via ScalarE activation and a
    DVE Schraudolph path) but in [128,512] half-tiles with a
    measured-rate 5:3 Scalar:DVE split, and iteration-0's mains are
    interleaved 8 chunks behind the build so they finish with it.
"""

import numpy as np
import ml_dtypes

C = 10          # classes
N = 8192        # points
S = 3           # spatial dims
R = 8           # cores
NL = N // R     # local points per core
KCH = N // 128  # 64 i-chunks
JCH = NL // 128  # 8 local j-chunks
NITER = 5
THETA_GAMMA = 8.0
DEG = 4         # spatial poly degree
M = 35          # monomials for DEG=4 in 3 vars
NGRP = 4        # PE column groups for the main matmul
CP = 32         # padded class dim for the main stationary

C1 = float(2**23) / float(np.log(2.0))
C2A = float(2**30)
GAMMA = 1.0406829  # E[(1+r)/2^r], r~U[0,1): Schraudolph mean ratio

_CACHE = {}


def _build_program():
    import concourse.mybir as mybir
    import concourse.tile as tile
    from concourse import bacc
    from concourse.bass import ts, ds

    f32 = mybir.dt.float32
    bf16 = mybir.dt.bfloat16
    fp8 = mybir.dt.float8e4
    i32 = mybir.dt.int32
    EXP = mybir.ActivationFunctionType.Exp

    nc = bacc.Bacc("TRN2", target_bir_lowering=False, debug=False, num_devices=R)

    # ---- I/O (host-side pre-transposed into row-contiguous layouts) ----
    ffa = nc.dram_tensor("ffa", [42, N], bf16, kind="ExternalInput")
    fla = nc.dram_tensor("fla", [42, NL], bf16, kind="ExternalInput")
    psiTl = nc.dram_tensor("psiTl", [128, JCH, M], bf16, kind="ExternalInput")
    psil = nc.dram_tensor("psil", [M, NL], bf16, kind="ExternalInput")
    pb_init = nc.dram_tensor("pb_init", [128, KCH, CP], fp8, kind="ExternalInput")
    wt0_hi = nc.dram_tensor("wt0_hi", [M, C], bf16, kind="ExternalInput")
    wt0_lo = nc.dram_tensor("wt0_lo", [M, C], bf16, kind="ExternalInput")
    unT_loc = nc.dram_tensor("unT_loc", [128, JCH, C], f32, kind="ExternalInput")
    m2 = nc.dram_tensor("m2", [4 * C, 4 * CP + 4 * C], bf16, kind="ExternalInput")
    idn = nc.dram_tensor("idn", [128, 128], bf16, kind="ExternalInput")
    sel = nc.dram_tensor("sel", [128, C], bf16, kind="ExternalInput")
    qT_out = nc.dram_tensor("qT_out", [128, JCH, C], f32, kind="ExternalOutput")

    b_act = -C2A / C1 + float(np.log(2.0)) + float(np.log(GAMMA))

    with tile.TileContext(nc) as tc:
        with (
            tc.tile_pool(name="const", bufs=1) as const,
            tc.tile_pool(name="state", bufs=1) as state,
            tc.tile_pool(name="spool", bufs=4) as spool,
            tc.tile_pool(name="opool", bufs=2) as opool,
            tc.tile_pool(name="qpool", bufs=4) as qpool,
            tc.tile_pool(name="psY", bufs=2, space="PSUM") as psY,
            tc.tile_pool(name="psP", bufs=1, space="PSUM") as psP,
            tc.tile_pool(name="psS", bufs=1, space="PSUM") as psS,
            tc.tile_pool(name="dram", bufs=4, space="DRAM") as dram,
        ):
            # ---- constants (loads split across engine DMA queues) ----
            ffa_sb = const.tile([42, N], bf16, name="ffa_sb")
            fla_sb = const.tile([42, NL], bf16, name="fla_sb")
            HN = N // 2
            nc.sync.dma_start(fla_sb[:], fla[:])
            nc.sync.dma_start(ffa_sb[0:10, 0:HN], ffa[0:10, 0:HN])
            nc.scalar.dma_start(ffa_sb[32:42, 0:HN], ffa[32:42, 0:HN])
            nc.sync.dma_start(ffa_sb[0:10, HN:N], ffa[0:10, HN:N])
            nc.scalar.dma_start(ffa_sb[32:42, HN:N], ffa[32:42, HN:N])
            psiTl_sb = const.tile([128, JCH, M], bf16, name="psiTl_sb")
            psil_sb = const.tile([M, NL], bf16, name="psil_sb")
            m2_sb = const.tile([4 * C, 4 * CP + 4 * C], bf16, name="m2_sb")
            idn_sb = const.tile([128, 128], bf16, name="idn_sb")
            sel_sb = const.tile([128, C], bf16, name="sel_sb")
            un_sb = const.tile([128, JCH, C], f32, name="un_sb")
            nc.gpsimd.dma_start(psiTl_sb[:], psiTl[:])
            nc.gpsimd.dma_start(psil_sb[:], psil[:])
            nc.gpsimd.dma_start(m2_sb[:], m2[:])
            nc.gpsimd.dma_start(idn_sb[:], idn[:])
            nc.gpsimd.dma_start(sel_sb[:], sel[:])
            nc.gpsimd.dma_start(un_sb[:], unT_loc[:])

            # warm up the collective stack early: pays the first-trigger
            # firmware cost + absorbs launch skew under the slab build
            bi0 = dram.tile([1, 16], bf16, name="bi0")
            bo0 = dram.tile([R, 16], bf16, addr_space="Shared", name="bo0")
            z_sb = const.tile([1, 16], bf16, name="z_sb")
            nc.gpsimd.memset(z_sb[:], 0)
            nc.gpsimd.dma_start(bi0[:], z_sb[:])
            nc.gpsimd.collective_compute(
                "AllGather",
                mybir.AluOpType.bypass,
                replica_groups=[list(range(R))],
                ins=[bi0[:].opt()],
                outs=[bo0[:].opt()],
            )

            # bilateral slab, fp8, SBUF-resident
            slab = const.tile([128, KCH, NL], fp8, name="slab")

            bact_sb = const.tile([128, 1], f32, name="bact_sb")
            nc.gpsimd.memset(bact_sb[:], b_act)

            # premixed bilateral distribution, fp8, zero-padded to CP cols
            pb8_sb = state.tile([128, KCH, CP], fp8, name="pb8_sb")
            nc.sync.dma_start(pb8_sb[:], pb_init[:])
            pb84 = pb8_sb[:].rearrange("p (r y) c -> p r y c", r=R)

            # spatial filter weights (hi/lo bf16 split of f32)
            wt_hi = state.tile([M, C], bf16, name="wt_hi")
            wt_lo = state.tile([M, C], bf16, name="wt_lo")
            nc.scalar.dma_start(wt_hi[:], wt0_hi[:])
            nc.scalar.dma_start(wt_lo[:], wt0_lo[:])
            # gathered W partials (16 = 8 ranks x 2 halves), f32
            wparts_sb = state.tile([M, 2 * R, C], f32, name="wparts_sb")
            w1_sb = state.tile([M, R, C], f32, name="w1_sb")
            w2_sb = state.tile([M, 4, C], f32, name="w2_sb")
            w3_sb = state.tile([M, 2, C], f32, name="w3_sb")
            wf_sb = state.tile([M, 1, C], f32, name="wf_sb")

            # softmax scratch (per half: [128, 4])
            mx_sb = state.tile([128, JCH], f32, name="mx_sb")
            sm_sb = state.tile([128, JCH], f32, name="sm_sb")
            rs_sb = state.tile([128, JCH], f32, name="rs_sb")
            el_sb = state.tile([128, JCH, C], f32, name="el_sb")

            # k-chunk orders: AG-half-0 chunks (y 0..3) then half-1
            ks_h0 = [r * JCH + y for y in range(4) for r in range(R)]
            ks_h1 = [r * JCH + y for y in range(4, JCH) for r in range(R)]
            ks_ag = ks_h0 + ks_h1

            def emit_mains(po, jh, ks):
                """64 accumulation matmuls into po's 4 column bands."""
                seen = set()
                last = {}
                for k in ks:
                    last[k % NGRP] = k
                jsl = ds(jh * 512, 512)
                for k in ks:
                    g = k % NGRP
                    nc.tensor.matmul(
                        po[32 * g:32 * g + CP, :],
                        pb8_sb[:, k, :],
                        slab[:, k, jsl],
                        tile_position=(0, 32 * g),
                        start=(g not in seen),
                        # band 0 is closed later by the osp accumulation
                        stop=(last[g] == k and g != 0),
                    )
                    seen.add(g)

            def emit_osp(po, jh):
                jsl = ds(jh * 512, 512)
                nc.tensor.matmul(
                    po[0:C, :], wt_hi[:], psil_sb[:, jsl],
                    tile_position=(0, 0), start=False, stop=False,
                )
                nc.tensor.matmul(
                    po[0:C, :], wt_lo[:], psil_sb[:, jsl],
                    tile_position=(0, 0), start=False, stop=True,
                )

            def emit_post(po, jh, t):
                jr = ds(4 * jh, 4)
                # PSUM -> SBUF bf16, split across Scalar and Vector
                poS = opool.tile([128, 512], bf16, name="poS")
                nc.scalar.copy(poS[:, 0:256], po[:, 0:256])
                nc.vector.tensor_scalar_add(poS[:, 256:512], po[:, 256:512], 0.0)
                # 4-band sum via sel matmul, straight into [j, c] layout
                qa = psS.tile([128, 4, C], f32, name="qa")
                for j in range(4):
                    nc.tensor.matmul(
                        qa[:, j, :], poS[:, ts(j, 128)], sel_sb[:],
                        start=True, stop=True,
                    )
                ql = qpool.tile([128, 4, C], f32, name="ql")
                nc.vector.tensor_add(ql[:], qa[:], un_sb[:, jr, :])
                if t == NITER - 1:
                    nc.sync.dma_start(qT_out[:, jr, :], ql[:])
                    return
                # ---- softmax over classes (free axis) ----
                hsl = ds(4 * jh, 4)
                nc.vector.reduce_max(
                    mx_sb[:, hsl], ql[:], axis=mybir.AxisListType.X
                )
                mx_b = mx_sb[:, hsl].unsqueeze(2).broadcast_to((128, 4, C))
                nc.vector.tensor_sub(el_sb[:, hsl, :], ql[:], mx_b)
                nc.scalar.activation(
                    el_sb[:, hsl, :], el_sb[:, hsl, :], EXP, bias=0.0, scale=1.0,
                )
                nc.vector.reduce_sum(
                    sm_sb[:, hsl], el_sb[:, hsl, :], axis=mybir.AxisListType.X
                )
                nc.vector.reciprocal(rs_sb[:, hsl], sm_sb[:, hsl])
                rs_b = rs_sb[:, hsl].unsqueeze(2).broadcast_to((128, 4, C))
                pl = qpool.tile([128, 4, C], bf16, name="pl")
                nc.vector.tensor_mul(pl[:], el_sb[:, hsl, :], rs_b)
                # ---- premix: pl^T, then block-diag mix -> [pb | ps] ----
                plT = psS.tile([4 * C, 128], f32, name="plT")
                nc.tensor.matmul(
                    plT[:], pl[:].rearrange("p y c -> p (y c)"), idn_sb[:],
                    start=True, stop=True,
                )
                plTs = spool.tile([4 * C, 128], bf16, name="plTs")
                nc.scalar.copy(plTs[:], plT[:])
                pbps = psS.tile([128, 4 * CP + 4 * C], f32, name="pbps")
                nc.tensor.matmul(
                    pbps[:], plTs[:], m2_sb[:], start=True, stop=True,
                )
                agp = spool.tile([128, 4 * CP + 4 * C], fp8, name="agp")
                nc.scalar.copy(agp[:], pbps[:])
                # ---- local spatial partial W = Psi_loc @ ps_loc^T ----
                wp = psS.tile([M, C], f32, name="wp")
                for j in range(4):
                    nc.tensor.matmul(
                        wp[:], psiTl_sb[:, 4 * jh + j, :],
                        agp[:, 4 * CP + C * j:4 * CP + C * j + C],
                        start=(j == 0), stop=(j == 3),
                    )
                w_sb = spool.tile([M, C], f32, name="w_sb")
                nc.vector.tensor_scalar_add(w_sb[:], wp[:], 0.0)
                # ---- bounce + AllGather (pb fp8 cols 0:40, W f32 bytes 40:80) ----
                bi = dram.tile([128, 4 * CP + 4 * C], fp8, name="bi")
                bo = dram.tile([R, 128, 4 * CP + 4 * C], fp8, addr_space="Shared", name="bo")
                eng = nc.sync if jh == 0 else nc.gpsimd
                eng.dma_start(bi[:, 0:4 * CP], agp[:, 0:4 * CP])
                eng.dma_start(
                    bi[0:M, 4 * CP:4 * CP + 4 * C],
                    w_sb[:].bitcast(fp8),
                )
                nc.gpsimd.collective_compute(
                    "AllGather",
                    mybir.AluOpType.bypass,
                    replica_groups=[list(range(R))],
                    ins=[bi[:].opt()],
                    outs=[bo[:].opt()],
                )
                # ---- scatter: pb chunks + W partials ----
                eng.dma_start(
                    pb84[:, :, 4 * jh:4 * jh + 4, :],
                    bo[:, :, 0:4 * CP].rearrange("r p (y c) -> p r y c", c=CP),
                )
                eng.dma_start(
                    wparts_sb[:, R * jh:R * jh + R, :].bitcast(fp8),
                    bo[:, 0:M, 4 * CP:4 * CP + 4 * C].rearrange("r p f -> p r f"),
                )
                if jh == 1:
                    # f32 tree-sum of the 16 W partials -> next iteration's wt
                    nc.vector.tensor_add(
                        w1_sb[:], wparts_sb[:, 0:R, :], wparts_sb[:, R:2 * R, :]
                    )
                    nc.vector.tensor_add(
                        w2_sb[:], w1_sb[:, 0:4, :], w1_sb[:, 4:8, :]
                    )
                    nc.vector.tensor_add(
                        w3_sb[:], w2_sb[:, 0:2, :], w2_sb[:, 2:4, :]
                    )
                    nc.vector.tensor_add(
                        wf_sb[:], w3_sb[:, 0:1, :], w3_sb[:, 1:2, :]
                    )
                    nc.scalar.copy(wt_hi[:], wf_sb[:, 0, :])
                    nc.vector.tensor_sub(wt_lo[:], wf_sb[:, 0, :], wt_hi[:])

            for t in range(NITER):
                po0 = psP.tile([128, 512], f32, name="po0")
                po1 = psP.tile([128, 512], f32, name="po1")
                if t == 0:
                    # slab build interleaved with iteration-0 mains (LAG chunks)
                    LAG = 8
                    seen0, seen1 = set(), set()
                    for kk in range(KCH + LAG):
                        if kk < KCH:
                            k = kk
                            rs = 32 * (k % 2)
                            for sh in range(2):
                                yt = psY.tile([128, 512], f32, name="yt")
                                nc.tensor.matmul(
                                    yt[:],
                                    ffa_sb[rs:rs + 10, ts(k, 128)],
                                    fla_sb[rs:rs + 10, ds(sh * 512, 512)],
                                    start=True, stop=True,
                                    tile_position=(rs, 0),
                                )
                                u = 2 * k + sh
                                ssl = ds(sh * 512, 512)
                                if u % 8 < 5:
                                    nc.scalar.activation(
                                        slab[:, k, ssl], yt[:], EXP,
                                        bias=bact_sb[:], scale=1.0 / C1,
                                    )
                                else:
                                    sc = spool.tile([128, 512], i32, name="sc")
                                    nc.vector.tensor_scalar_add(sc[:], yt[:], 0.0)
                                    nc.vector.tensor_scalar_add(
                                        slab[:, k, ssl], sc[:].bitcast(f32), 0.0
                                    )
                        if kk >= LAG:
                            k = kk - LAG
                            g = k % NGRP
                            for po, jh, seen in ((po0, 0, seen0), (po1, 1, seen1)):
                                nc.tensor.matmul(
                                    po[32 * g:32 * g + CP, :],
                                    pb8_sb[:, k, :],
                                    slab[:, k, ds(jh * 512, 512)],
                                    tile_position=(0, 32 * g),
                                    start=(g not in seen),
                                    stop=(k >= KCH - NGRP and g != 0),
                                )
                                seen.add(g)
                    emit_osp(po0, 0)
                    emit_post(po0, 0, t)
                    emit_osp(po1, 1)
                    emit_post(po1, 1, t)
                else:
                    emit_mains(po0, 0, ks_ag)
                    emit_osp(po0, 0)
                    emit_post(po0, 0, t)
                    emit_mains(po1, 1, ks_ag)
                    emit_osp(po1, 1)
                    emit_post(po1, 1, t)

    nc.compile()
    return nc


def _get_program():
    if "nc" not in _CACHE:
        _CACHE["nc"] = _build_program()
    return _CACHE["nc"]


def _host_prep(unaries, feat, sw, bw, compat):
    bf = ml_dtypes.bfloat16
    f8 = ml_dtypes.float8_e4m3
    f = feat.astype(np.float32)
    f2 = np.sum(f * f, axis=0)

    sqc = np.float32(np.sqrt(C1))
    fr = (sqc * f).astype(bf)                      # [6, N] bf16 scaled features
    r_row = (np.float32(C1) * (-0.5 * f2)).astype(bf)   # bf16 |f|^2 row

    # exact correction for the bf16 rounding of the j-side row, folded
    # into the exponent as one extra augmented row
    r_used = r_row.astype(np.float32)
    v_row = (r_used + np.float32(C1) * (0.5 * f2).astype(np.float32)).astype(bf)

    # i-side rows (lhsT): [sq*f(6); r_i; 1; 1; 1],
    # j-side rows (rhs):  [sq*f(6); 1; r_j; 2^30; v]
    ffa = np.zeros((42, N), dtype=bf)
    fla_full = np.zeros((42, N), dtype=bf)
    for off in (0, 32):
        ffa[off:off + 6] = fr
        ffa[off + 6] = r_row
        ffa[off + 7] = bf(1.0)
        ffa[off + 8] = bf(1.0)
        ffa[off + 9] = bf(1.0)
        fla_full[off:off + 6] = fr
        fla_full[off + 6] = bf(1.0)
        fla_full[off + 7] = r_row
        fla_full[off + 8] = bf(C2A)
        fla_full[off + 9] = v_row

    # spatial poly features
    from math import factorial
    s = f[:S] / np.float32(THETA_GAMMA)
    a_sp = np.exp(-0.5 * np.sum(s * s, axis=0))
    rows = []
    for a in range(DEG + 1):
        for b in range(DEG + 1 - a):
            for c in range(DEG + 1 - a - b):
                coef = 1.0 / np.sqrt(factorial(a) * factorial(b) * factorial(c))
                rows.append(coef * s[0] ** a * s[1] ** b * s[2] ** c)
    psi = (np.stack(rows) * a_sp[None, :]).astype(bf)    # [M, N]
    # local psi in i-layout per core: [128, JCH, M]
    psiT = np.ascontiguousarray(
        psi.T.reshape(KCH, 128, M).transpose(1, 0, 2)
    )  # [128, KCH, M]

    # premix matrices
    Mbl = -(compat @ bw)
    Msp = -(compat @ sw)
    Mbl_s = (Mbl / np.float32(2.0 * GAMMA)).astype(bf)
    Msp_s = Msp.astype(bf)

    # block-diagonal mix matrix [40, 168]: rows (j,c) ->
    # cols 0:128 = pb (4 x CP-padded blocks), cols 128:168 = ps (4 x C)
    m2 = np.zeros((4 * C, 4 * CP + 4 * C), dtype=bf)
    for j in range(4):
        m2[C * j:C * j + C, CP * j:CP * j + C] = Mbl_s.T
        m2[C * j:C * j + C, 4 * CP + C * j:4 * CP + C * j + C] = Msp_s.T

    idn = np.eye(128, dtype=bf)
    sel = np.zeros((128, C), dtype=bf)
    for g in range(NGRP):
        for c in range(C):
            sel[32 * g + c, c] = bf(1.0)

    # iteration-0 distributions (host softmax + premix)
    mx = unaries.max(axis=0, keepdims=True)
    e = np.exp(unaries - mx, dtype=np.float32)
    p0 = e / e.sum(axis=0, keepdims=True)
    p0 = p0.T.astype(bf).astype(np.float32).T       # device pl is bf16
    pb0 = (Mbl_s.astype(np.float32) @ p0).astype(f8)     # [C, N]
    ps0 = (Msp_s.astype(np.float32) @ p0).astype(f8)
    wt0 = psi.astype(np.float32) @ ps0.astype(np.float32).T   # [M, C] f32
    wt0_hi = wt0.astype(bf)
    wt0_lo = (wt0 - wt0_hi.astype(np.float32)).astype(bf)
    # [128, KCH, CP]: pb0T[p, k, c] = pb0[c, 128k+p], zero-padded
    pb0T = np.zeros((128, KCH, CP), dtype=f8)
    pb0T[:, :, 0:C] = pb0.T.reshape(KCH, 128, C).transpose(1, 0, 2)

    qT_init = np.ascontiguousarray(unaries.T).astype(np.float32)
    return (ffa, fla_full, psiT, psi, m2, idn, sel,
            pb0T, wt0_hi, wt0_lo, qT_init)


def _make_in_maps(inputs):
    unaries = np.asarray(inputs["unaries"], dtype=np.float32)
    feat = np.asarray(inputs["feat"], dtype=np.float32)
    sw = np.asarray(inputs["spatial_weights"], dtype=np.float32)
    bw = np.asarray(inputs["bilateral_weights"], dtype=np.float32)
    compat = np.asarray(inputs["compatibility_matrix"], dtype=np.float32)

    (ffa, fla_full, psiT, psi, m2, idn, sel,
     pb0T, wt0_hi, wt0_lo, qT_init) = _host_prep(unaries, feat, sw, bw, compat)
    in_maps = []
    for r in range(R):
        jsl = slice(r * NL, (r + 1) * NL)
        ksl = slice(r * JCH, (r + 1) * JCH)
        in_maps.append({
            "ffa": ffa,
            "fla": np.ascontiguousarray(fla_full[:, jsl]),
            "psiTl": np.ascontiguousarray(psiT[:, ksl, :]),
            "psil": np.ascontiguousarray(psi[:, jsl]),
            "pb_init": pb0T,
            "wt0_hi": wt0_hi,
            "wt0_lo": wt0_lo,
            "unT_loc": np.ascontiguousarray(
                qT_init[jsl].reshape(JCH, 128, C).transpose(1, 0, 2)
            ),
            "m2": m2,
            "idn": idn,
            "sel": sel,
        })
    return in_maps


def kernel(unaries, feat, spatial_weights, bilateral_weights, compatibility_matrix):
    from concourse.bass_utils import run_bass_kernel_spmd

    in_maps = _make_in_maps({
        "unaries": unaries,
        "feat": feat,
        "spatial_weights": spatial_weights,
        "bilateral_weights": bilateral_weights,
        "compatibility_matrix": compatibility_matrix,
    })
    nc = _get_program()
    res = run_bass_kernel_spmd(nc, in_maps, core_ids=list(range(R)))

    q = np.empty((C, N), dtype=np.float32)
    for r in range(R):
        out = res.results[r]["qT_out"]          # [128, JCH, C]
        q[:, r * NL:(r + 1) * NL] = out.transpose(2, 1, 0).reshape(C, NL)
    return q


# revision 10
# speedup vs baseline: 1.1828x; 1.1828x over previous
"""CRF mean-field (nn_CRF) Trainium2 kernel, SPMD over 8 NeuronCores. v3.

Math: 5 iterations of
    p   = softmax(q, axis=classes)
    q   = unaries - compat @ (sw @ (p @ K_sp) + bw @ (p @ K_bl))

Design (v3, rewritten from v2 to shorten the per-iteration critical path):

  * The CxC mixing commutes with the N-axis filtering, so the mixing is
    PRE-applied to the local p shard before the AllGather:
        pb = (-(compat@bw)/(2*gamma)) @ p   (feeds the bilateral slab matmul)
        ps = (-(compat@sw)) @ p             (feeds the spatial poly path)
    via two tiny PE matmuls (transpose against identity, then a
    block-diagonal [40,80] mix matrix). Only pb is AllGathered; the
    spatial path ships as a per-core partial W_r = Psi_loc @ ps_loc^T
    (f32, byte-packed into the same AG buffer) and the 16 partials are
    tree-summed post-AG. This removes the per-iteration 64-chunk wtp
    accumulation and all post-filter mixing matmuls.

  * Main bilateral matmul: pb8 [128,32] fp8 stationary (zero-padded
    cols 10-31 so every PSUM row is defined) x slab [128,512] fp8
    moving, 4 PE column groups. The 4 band partials + the spatial osp
    (accumulated into band 0) are summed by ONE [128,128]x[128,10]
    sel-matmul per j-chunk after a single PSUM->SBUF bf16 copy --
    replacing v2's serial 2.7us DVE combine chain per half.

  * AllGathers (2/iter, 10KB) overlap compute: the k-loop is ordered
    AG0-chunks then AG1-chunks, and the j-half-1 mains + post run under
    AG0's flight.

  * Slab build unchanged numerically (exp via ScalarE activation and a
    DVE Schraudolph path) but in [128,512] half-tiles with a
    measured-rate 5:3 Scalar:DVE split, and iteration-0's mains are
    interleaved 8 chunks behind the build so they finish with it.
"""

import numpy as np
import ml_dtypes

C = 10          # classes
N = 8192        # points
S = 3           # spatial dims
R = 8           # cores
NL = N // R     # local points per core
KCH = N // 128  # 64 i-chunks
JCH = NL // 128  # 8 local j-chunks
NITER = 5
THETA_GAMMA = 8.0
DEG = 4         # spatial poly degree
M = 35          # monomials for DEG=4 in 3 vars
NGRP = 4        # PE column groups for the main matmul
CP = 32         # padded class dim for the main stationary

C1 = float(2**23) / float(np.log(2.0))
C2A = float(2**30)
GAMMA = 1.0406829  # E[(1+r)/2^r], r~U[0,1): Schraudolph mean ratio

_CACHE = {}


def _build_program():
    import concourse.mybir as mybir
    import concourse.tile as tile
    from concourse import bacc
    from concourse.bass import ts, ds

    f32 = mybir.dt.float32
    bf16 = mybir.dt.bfloat16
    fp8 = mybir.dt.float8e4
    i32 = mybir.dt.int32
    EXP = mybir.ActivationFunctionType.Exp

    nc = bacc.Bacc("TRN2", target_bir_lowering=False, debug=False, num_devices=R)

    # ---- I/O (host-side pre-transposed into row-contiguous layouts) ----
    ffa = nc.dram_tensor("ffa", [42, N], bf16, kind="ExternalInput")
    fla = nc.dram_tensor("fla", [42, NL], bf16, kind="ExternalInput")
    psiTl = nc.dram_tensor("psiTl", [128, JCH, M], bf16, kind="ExternalInput")
    psil = nc.dram_tensor("psil", [M, NL], bf16, kind="ExternalInput")
    pb_init = nc.dram_tensor("pb_init", [128, KCH, CP], fp8, kind="ExternalInput")
    wt0_hi = nc.dram_tensor("wt0_hi", [M, C], bf16, kind="ExternalInput")
    wt0_lo = nc.dram_tensor("wt0_lo", [M, C], bf16, kind="ExternalInput")
    unT_loc = nc.dram_tensor("unT_loc", [128, JCH, C], f32, kind="ExternalInput")
    m2 = nc.dram_tensor("m2", [4 * C, 4 * CP + 4 * C], bf16, kind="ExternalInput")
    idn = nc.dram_tensor("idn", [128, 128], bf16, kind="ExternalInput")
    sel = nc.dram_tensor("sel", [128, C], bf16, kind="ExternalInput")
    qT_out = nc.dram_tensor("qT_out", [128, JCH, C], f32, kind="ExternalOutput")

    b_act = -C2A / C1 + float(np.log(2.0)) + float(np.log(GAMMA))

    with tile.TileContext(nc) as tc:
        with (
            tc.tile_pool(name="const", bufs=1) as const,
            tc.tile_pool(name="state", bufs=1) as state,
            tc.tile_pool(name="spool", bufs=4) as spool,
            tc.tile_pool(name="opool", bufs=2) as opool,
            tc.tile_pool(name="qpool", bufs=4) as qpool,
            tc.tile_pool(name="psY", bufs=4, space="PSUM") as psY,
            tc.tile_pool(name="psP", bufs=1, space="PSUM") as psP,
            tc.tile_pool(name="psS", bufs=1, space="PSUM") as psS,
            tc.tile_pool(name="dram", bufs=4, space="DRAM") as dram,
        ):
            # ---- constants (loads split across engine DMA queues) ----
            ffa_sb = const.tile([42, N], bf16, name="ffa_sb")
            fla_sb = const.tile([42, NL], bf16, name="fla_sb")
            HN = N // 2
            nc.sync.dma_start(fla_sb[:], fla[:])
            nc.sync.dma_start(ffa_sb[0:10, 0:HN], ffa[0:10, 0:HN])
            nc.scalar.dma_start(ffa_sb[32:42, 0:HN], ffa[32:42, 0:HN])
            nc.sync.dma_start(ffa_sb[0:10, HN:N], ffa[0:10, HN:N])
            nc.scalar.dma_start(ffa_sb[32:42, HN:N], ffa[32:42, HN:N])
            psiTl_sb = const.tile([128, JCH, M], bf16, name="psiTl_sb")
            psil_sb = const.tile([M, NL], bf16, name="psil_sb")
            m2_sb = const.tile([4 * C, 4 * CP + 4 * C], bf16, name="m2_sb")
            idn_sb = const.tile([128, 128], bf16, name="idn_sb")
            sel_sb = const.tile([128, C], bf16, name="sel_sb")
            un_sb = const.tile([128, JCH, C], f32, name="un_sb")
            nc.gpsimd.dma_start(psiTl_sb[:], psiTl[:])
            nc.gpsimd.dma_start(psil_sb[:], psil[:])
            nc.gpsimd.dma_start(m2_sb[:], m2[:])
            nc.gpsimd.dma_start(idn_sb[:], idn[:])
            nc.gpsimd.dma_start(sel_sb[:], sel[:])
            nc.gpsimd.dma_start(un_sb[:], unT_loc[:])

            # warm up the collective stack early: pays the first-trigger
            # firmware cost + absorbs launch skew under the slab build
            bi0 = dram.tile([128, 4 * CP + 4 * C], fp8, name="bi0")
            bo0 = dram.tile([R, 128, 4 * CP + 4 * C], fp8, addr_space="Shared", name="bo0")
            z_sb = const.tile([128, 4 * CP + 4 * C], fp8, name="z_sb")
            nc.gpsimd.memset(z_sb[:], 0)
            nc.gpsimd.dma_start(bi0[:], z_sb[:])
            nc.gpsimd.collective_compute(
                "AllGather",
                mybir.AluOpType.bypass,
                replica_groups=[list(range(R))],
                ins=[bi0[:].opt()],
                outs=[bo0[:].opt()],
            )

            # bilateral slab, fp8, SBUF-resident
            slab = const.tile([128, KCH, NL], fp8, name="slab")

            bact_sb = const.tile([128, 1], f32, name="bact_sb")
            nc.gpsimd.memset(bact_sb[:], b_act)

            # premixed bilateral distribution, fp8, zero-padded to CP cols
            pb8_sb = state.tile([128, KCH, CP], fp8, name="pb8_sb")
            nc.sync.dma_start(pb8_sb[:], pb_init[:])
            pb84 = pb8_sb[:].rearrange("p (r y) c -> p r y c", r=R)

            # spatial filter weights (hi/lo bf16 split of f32)
            wt_hi = state.tile([M, C], bf16, name="wt_hi")
            wt_lo = state.tile([M, C], bf16, name="wt_lo")
            nc.scalar.dma_start(wt_hi[:], wt0_hi[:])
            nc.scalar.dma_start(wt_lo[:], wt0_lo[:])
            # gathered W partials (16 = 8 ranks x 2 halves), f32
            wparts_sb = state.tile([M, 2 * R, C], f32, name="wparts_sb")
            w1_sb = state.tile([M, R, C], f32, name="w1_sb")
            w2_sb = state.tile([M, 4, C], f32, name="w2_sb")
            w3_sb = state.tile([M, 2, C], f32, name="w3_sb")
            wf_sb = state.tile([M, 1, C], f32, name="wf_sb")

            # softmax scratch (per half: [128, 4])
            mx_sb = state.tile([128, JCH], f32, name="mx_sb")
            sm_sb = state.tile([128, JCH], f32, name="sm_sb")
            rs_sb = state.tile([128, JCH], f32, name="rs_sb")
            el_sb = state.tile([128, JCH, C], f32, name="el_sb")

            # k-chunk orders: AG-half-0 chunks (y 0..3) then half-1
            ks_h0 = [r * JCH + y for y in range(4) for r in range(R)]
            ks_h1 = [r * JCH + y for y in range(4, JCH) for r in range(R)]
            ks_ag = ks_h0 + ks_h1

            def emit_mains(po, jh, ks):
                """64 accumulation matmuls into po's 4 column bands."""
                seen = set()
                last = {}
                for k in ks:
                    last[k % NGRP] = k
                jsl = ds(jh * 512, 512)
                for k in ks:
                    g = k % NGRP
                    nc.tensor.matmul(
                        po[32 * g:32 * g + CP, :],
                        pb8_sb[:, k, :],
                        slab[:, k, jsl],
                        tile_position=(0, 32 * g),
                        start=(g not in seen),
                        # band 0 is closed later by the osp accumulation
                        stop=(last[g] == k and g != 0),
                    )
                    seen.add(g)

            def emit_osp(po, jh):
                jsl = ds(jh * 512, 512)
                nc.tensor.matmul(
                    po[0:C, :], wt_hi[:], psil_sb[:, jsl],
                    tile_position=(0, 0), start=False, stop=False,
                )
                nc.tensor.matmul(
                    po[0:C, :], wt_lo[:], psil_sb[:, jsl],
                    tile_position=(0, 0), start=False, stop=True,
                )

            def emit_post(po, jh, t):
                jr = ds(4 * jh, 4)
                # PSUM -> SBUF bf16, split across Scalar and Vector
                poS = opool.tile([128, 512], bf16, name="poS")
                nc.scalar.copy(poS[:, 0:256], po[:, 0:256])
                nc.vector.tensor_scalar_add(poS[:, 256:512], po[:, 256:512], 0.0)
                # 4-band sum via sel matmul, straight into [j, c] layout
                qw = psS.tile([128, 4 * C + C + 2], f32, name="qw")
                for j in range(4):
                    nc.tensor.matmul(
                        qw[:, C * j:C * j + C], poS[:, ts(j, 128)], sel_sb[:],
                        start=True, stop=True,
                    )
                ql = qpool.tile([128, 4, C], f32, name="ql")
                nc.vector.tensor_add(
                    ql[:],
                    qw[:, 0:4 * C].rearrange("p (y c) -> p y c", c=C),
                    un_sb[:, jr, :],
                )
                if t == NITER - 1:
                    nc.sync.dma_start(qT_out[:, jr, :], ql[:])
                    return
                # ---- softmax over classes (free axis) ----
                hsl = ds(4 * jh, 4)
                nc.vector.reduce_max(
                    mx_sb[:, hsl], ql[:], axis=mybir.AxisListType.X
                )
                mx_b = mx_sb[:, hsl].unsqueeze(2).broadcast_to((128, 4, C))
                nc.vector.tensor_sub(el_sb[:, hsl, :], ql[:], mx_b)
                nc.scalar.activation(
                    el_sb[:, hsl, :], el_sb[:, hsl, :], EXP, bias=0.0, scale=1.0,
                )
                nc.vector.reduce_sum(
                    sm_sb[:, hsl], el_sb[:, hsl, :], axis=mybir.AxisListType.X
                )
                nc.vector.reciprocal(rs_sb[:, hsl], sm_sb[:, hsl])
                rs_b = rs_sb[:, hsl].unsqueeze(2).broadcast_to((128, 4, C))
                pl = qpool.tile([128, 4, C], bf16, name="pl")
                nc.vector.tensor_mul(pl[:], el_sb[:, hsl, :], rs_b)
                # ---- premix: pl^T, then block-diag mix -> [pb | ps] ----
                pmx = psS.tile([128, 4 * CP + 4 * C], f32, name="pmx")
                plT = pmx[0:4 * C, 0:128]
                nc.tensor.matmul(
                    plT, pl[:].rearrange("p y c -> p (y c)"), idn_sb[:],
                    start=True, stop=True,
                )
                plTs = spool.tile([4 * C, 128], bf16, name="plTs")
                nc.scalar.copy(plTs[:], plT)
                pbps = pmx[:, :]
                nc.tensor.matmul(
                    pbps, plTs[:], m2_sb[:], start=True, stop=True,
                )
                agp = spool.tile([128, 4 * CP + 4 * C], fp8, name="agp")
                nc.scalar.copy(agp[:], pbps)
                # ---- local spatial partial W = Psi_loc @ ps_loc^T ----
                wp = qw[0:M, 4 * C:4 * C + C]
                for j in range(4):
                    nc.tensor.matmul(
                        wp, psiTl_sb[:, 4 * jh + j, :],
                        agp[:, 4 * CP + C * j:4 * CP + C * j + C],
                        start=(j == 0), stop=(j == 3),
                    )
                w_sb = spool.tile([M, C], f32, name="w_sb")
                nc.vector.tensor_scalar_add(w_sb[:], wp, 0.0)
                # ---- bounce + AllGather (pb fp8 cols 0:40, W f32 bytes 40:80) ----
                bi = dram.tile([128, 4 * CP + 4 * C], fp8, name="bi")
                bo = dram.tile([R, 128, 4 * CP + 4 * C], fp8, addr_space="Shared", name="bo")
                eng = nc.sync if jh == 0 else nc.gpsimd
                eng.dma_start(bi[:, 0:4 * CP], agp[:, 0:4 * CP])
                eng.dma_start(
                    bi[0:M, 4 * CP:4 * CP + 4 * C],
                    w_sb[:].bitcast(fp8),
                )
                nc.gpsimd.collective_compute(
                    "AllGather",
                    mybir.AluOpType.bypass,
                    replica_groups=[list(range(R))],
                    ins=[bi[:].opt()],
                    outs=[bo[:].opt()],
                )
                # ---- scatter: pb chunks + W partials ----
                eng.dma_start(
                    pb84[:, :, 4 * jh:4 * jh + 4, :],
                    bo[:, :, 0:4 * CP].rearrange("r p (y c) -> p r y c", c=CP),
                )
                eng.dma_start(
                    wparts_sb[:, R * jh:R * jh + R, :].bitcast(fp8),
                    bo[:, 0:M, 4 * CP:4 * CP + 4 * C].rearrange("r p f -> p r f"),
                )
                if jh == 1:
                    # f32 tree-sum of the 16 W partials -> next iteration's wt
                    nc.vector.tensor_add(
                        w1_sb[:], wparts_sb[:, 0:R, :], wparts_sb[:, R:2 * R, :]
                    )
                    nc.vector.tensor_add(
                        w2_sb[:], w1_sb[:, 0:4, :], w1_sb[:, 4:8, :]
                    )
                    nc.vector.tensor_add(
                        w3_sb[:], w2_sb[:, 0:2, :], w2_sb[:, 2:4, :]
                    )
                    nc.vector.tensor_add(
                        wf_sb[:], w3_sb[:, 0:1, :], w3_sb[:, 1:2, :]
                    )
                    nc.scalar.copy(wt_hi[:], wf_sb[:, 0, :])
                    nc.vector.tensor_sub(wt_lo[:], wf_sb[:, 0, :], wt_hi[:])

            for t in range(NITER):
                po0 = psP.tile([128, 512], f32, name="po0")
                po1 = psP.tile([128, 512], f32, name="po1")
                if t == 0:
                    # slab build interleaved with iteration-0 mains (LAG chunks)
                    LAG = 8
                    seen0, seen1 = set(), set()
                    for kk in range(KCH + LAG):
                        if kk < KCH:
                            k = kk
                            rs = 32 * (k % 2)
                            for sh in range(2):
                                yt = psY.tile([128, 512], f32, name="yt")
                                nc.tensor.matmul(
                                    yt[:],
                                    ffa_sb[rs:rs + 10, ts(k, 128)],
                                    fla_sb[rs:rs + 10, ds(sh * 512, 512)],
                                    start=True, stop=True,
                                    tile_position=(rs, 0),
                                )
                                u = 2 * k + sh
                                ssl = ds(sh * 512, 512)
                                if u % 8 in (0, 2, 4, 6, 7):
                                    nc.scalar.activation(
                                        slab[:, k, ssl], yt[:], EXP,
                                        bias=bact_sb[:], scale=1.0 / C1,
                                    )
                                else:
                                    sc = spool.tile([128, 512], i32, name="sc")
                                    nc.vector.tensor_scalar_add(sc[:], yt[:], 0.0)
                                    nc.vector.tensor_scalar_add(
                                        slab[:, k, ssl], sc[:].bitcast(f32), 0.0
                                    )
                        if kk >= LAG:
                            k = kk - LAG
                            g = k % NGRP
                            for po, jh, seen in ((po0, 0, seen0), (po1, 1, seen1)):
                                nc.tensor.matmul(
                                    po[32 * g:32 * g + CP, :],
                                    pb8_sb[:, k, :],
                                    slab[:, k, ds(jh * 512, 512)],
                                    tile_position=(0, 32 * g),
                                    start=(g not in seen),
                                    stop=(k >= KCH - NGRP and g != 0),
                                )
                                seen.add(g)
                    emit_osp(po0, 0)
                    emit_osp(po1, 1)
                    emit_post(po0, 0, t)
                    emit_post(po1, 1, t)
                else:
                    seen = set()
                    last = {}
                    for k in ks_ag:
                        last[k % NGRP] = k
                    for k in ks_ag:
                        g = k % NGRP
                        for po, jh in ((po0, 0), (po1, 1)):
                            nc.tensor.matmul(
                                po[32 * g:32 * g + CP, :],
                                pb8_sb[:, k, :],
                                slab[:, k, ds(jh * 512, 512)],
                                tile_position=(0, 32 * g),
                                start=((g, jh) not in seen),
                                stop=(last[g] == k and g != 0),
                            )
                            seen.add((g, jh))
                    emit_osp(po0, 0)
                    emit_osp(po1, 1)
                    emit_post(po0, 0, t)
                    emit_post(po1, 1, t)

    nc.compile()
    return nc


def _get_program():
    if "nc" not in _CACHE:
        _CACHE["nc"] = _build_program()
    return _CACHE["nc"]


def _host_prep(unaries, feat, sw, bw, compat):
    bf = ml_dtypes.bfloat16
    f8 = ml_dtypes.float8_e4m3
    f = feat.astype(np.float32)
    f2 = np.sum(f * f, axis=0)

    sqc = np.float32(np.sqrt(C1))
    fr = (sqc * f).astype(bf)                      # [6, N] bf16 scaled features
    r_row = (np.float32(C1) * (-0.5 * f2)).astype(bf)   # bf16 |f|^2 row

    # exact correction for the bf16 rounding of the j-side row, folded
    # into the exponent as one extra augmented row
    r_used = r_row.astype(np.float32)
    v_row = (r_used + np.float32(C1) * (0.5 * f2).astype(np.float32)).astype(bf)

    # i-side rows (lhsT): [sq*f(6); r_i; 1; 1; 1],
    # j-side rows (rhs):  [sq*f(6); 1; r_j; 2^30; v]
    ffa = np.zeros((42, N), dtype=bf)
    fla_full = np.zeros((42, N), dtype=bf)
    for off in (0, 32):
        ffa[off:off + 6] = fr
        ffa[off + 6] = r_row
        ffa[off + 7] = bf(1.0)
        ffa[off + 8] = bf(1.0)
        ffa[off + 9] = bf(1.0)
        fla_full[off:off + 6] = fr
        fla_full[off + 6] = bf(1.0)
        fla_full[off + 7] = r_row
        fla_full[off + 8] = bf(C2A)
        fla_full[off + 9] = v_row

    # spatial poly features
    from math import factorial
    s = f[:S] / np.float32(THETA_GAMMA)
    a_sp = np.exp(-0.5 * np.sum(s * s, axis=0))
    rows = []
    for a in range(DEG + 1):
        for b in range(DEG + 1 - a):
            for c in range(DEG + 1 - a - b):
                coef = 1.0 / np.sqrt(factorial(a) * factorial(b) * factorial(c))
                rows.append(coef * s[0] ** a * s[1] ** b * s[2] ** c)
    psi = (np.stack(rows) * a_sp[None, :]).astype(bf)    # [M, N]
    # local psi in i-layout per core: [128, JCH, M]
    psiT = np.ascontiguousarray(
        psi.T.reshape(KCH, 128, M).transpose(1, 0, 2)
    )  # [128, KCH, M]

    # premix matrices
    Mbl = -(compat @ bw)
    Msp = -(compat @ sw)
    Mbl_s = (Mbl / np.float32(2.0 * GAMMA)).astype(bf)
    Msp_s = Msp.astype(bf)

    # block-diagonal mix matrix [40, 168]: rows (j,c) ->
    # cols 0:128 = pb (4 x CP-padded blocks), cols 128:168 = ps (4 x C)
    m2 = np.zeros((4 * C, 4 * CP + 4 * C), dtype=bf)
    for j in range(4):
        m2[C * j:C * j + C, CP * j:CP * j + C] = Mbl_s.T
        m2[C * j:C * j + C, 4 * CP + C * j:4 * CP + C * j + C] = Msp_s.T

    idn = np.eye(128, dtype=bf)
    sel = np.zeros((128, C), dtype=bf)
    for g in range(NGRP):
        for c in range(C):
            sel[32 * g + c, c] = bf(1.0)

    # iteration-0 distributions (host softmax + premix)
    mx = unaries.max(axis=0, keepdims=True)
    e = np.exp(unaries - mx, dtype=np.float32)
    p0 = e / e.sum(axis=0, keepdims=True)
    p0 = p0.T.astype(bf).astype(np.float32).T       # device pl is bf16
    pb0 = (Mbl_s.astype(np.float32) @ p0).astype(f8)     # [C, N]
    ps0 = (Msp_s.astype(np.float32) @ p0).astype(f8)
    wt0 = psi.astype(np.float32) @ ps0.astype(np.float32).T   # [M, C] f32
    wt0_hi = wt0.astype(bf)
    wt0_lo = (wt0 - wt0_hi.astype(np.float32)).astype(bf)
    # [128, KCH, CP]: pb0T[p, k, c] = pb0[c, 128k+p], zero-padded
    pb0T = np.zeros((128, KCH, CP), dtype=f8)
    pb0T[:, :, 0:C] = pb0.T.reshape(KCH, 128, C).transpose(1, 0, 2)

    qT_init = np.ascontiguousarray(unaries.T).astype(np.float32)
    return (ffa, fla_full, psiT, psi, m2, idn, sel,
            pb0T, wt0_hi, wt0_lo, qT_init)


def _make_in_maps(inputs):
    unaries = np.asarray(inputs["unaries"], dtype=np.float32)
    feat = np.asarray(inputs["feat"], dtype=np.float32)
    sw = np.asarray(inputs["spatial_weights"], dtype=np.float32)
    bw = np.asarray(inputs["bilateral_weights"], dtype=np.float32)
    compat = np.asarray(inputs["compatibility_matrix"], dtype=np.float32)

    (ffa, fla_full, psiT, psi, m2, idn, sel,
     pb0T, wt0_hi, wt0_lo, qT_init) = _host_prep(unaries, feat, sw, bw, compat)
    in_maps = []
    for r in range(R):
        jsl = slice(r * NL, (r + 1) * NL)
        ksl = slice(r * JCH, (r + 1) * JCH)
        in_maps.append({
            "ffa": ffa,
            "fla": np.ascontiguousarray(fla_full[:, jsl]),
            "psiTl": np.ascontiguousarray(psiT[:, ksl, :]),
            "psil": np.ascontiguousarray(psi[:, jsl]),
            "pb_init": pb0T,
            "wt0_hi": wt0_hi,
            "wt0_lo": wt0_lo,
            "unT_loc": np.ascontiguousarray(
                qT_init[jsl].reshape(JCH, 128, C).transpose(1, 0, 2)
            ),
            "m2": m2,
            "idn": idn,
            "sel": sel,
        })
    return in_maps


def kernel(unaries, feat, spatial_weights, bilateral_weights, compatibility_matrix):
    from concourse.bass_utils import run_bass_kernel_spmd

    in_maps = _make_in_maps({
        "unaries": unaries,
        "feat": feat,
        "spatial_weights": spatial_weights,
        "bilateral_weights": bilateral_weights,
        "compatibility_matrix": compatibility_matrix,
    })
    nc = _get_program()
    res = run_bass_kernel_spmd(nc, in_maps, core_ids=list(range(R)))

    q = np.empty((C, N), dtype=np.float32)
    for r in range(R):
        out = res.results[r]["qT_out"]          # [128, JCH, C]
        q[:, r * NL:(r + 1) * NL] = out.transpose(2, 1, 0).reshape(C, NL)
    return q


# revision 15
# speedup vs baseline: 1.2382x; 1.0469x over previous
"""CRF mean-field (nn_CRF) Trainium2 kernel, SPMD over 8 NeuronCores. v3.

Math: 5 iterations of
    p   = softmax(q, axis=classes)
    q   = unaries - compat @ (sw @ (p @ K_sp) + bw @ (p @ K_bl))

Design (v3, rewritten from v2 to shorten the per-iteration critical path):

  * The CxC mixing commutes with the N-axis filtering, so the mixing is
    PRE-applied to the local p shard before the AllGather:
        pb = (-(compat@bw)/(2*gamma)) @ p   (feeds the bilateral slab matmul)
        ps = (-(compat@sw)) @ p             (feeds the spatial poly path)
    via two tiny PE matmuls (transpose against identity, then a
    block-diagonal [40,80] mix matrix). Only pb is AllGathered; the
    spatial path ships as a per-core partial W_r = Psi_loc @ ps_loc^T
    (f32, byte-packed into the same AG buffer) and the 16 partials are
    tree-summed post-AG. This removes the per-iteration 64-chunk wtp
    accumulation and all post-filter mixing matmuls.

  * Main bilateral matmul: pb8 [128,32] fp8 stationary (zero-padded
    cols 10-31 so every PSUM row is defined) x slab [128,512] fp8
    moving, 4 PE column groups. The 4 band partials + the spatial osp
    (accumulated into band 0) are summed by ONE [128,128]x[128,10]
    sel-matmul per j-chunk after a single PSUM->SBUF bf16 copy --
    replacing v2's serial 2.7us DVE combine chain per half.

  * AllGathers (2/iter, 10KB) overlap compute: the k-loop is ordered
    AG0-chunks then AG1-chunks, and the j-half-1 mains + post run under
    AG0's flight.

  * Slab build unchanged numerically (exp via ScalarE activation and a
    DVE Schraudolph path) but in [128,512] half-tiles with a
    measured-rate 5:3 Scalar:DVE split, and iteration-0's mains are
    interleaved 8 chunks behind the build so they finish with it.
"""

import numpy as np
import ml_dtypes

C = 10          # classes
N = 8192        # points
S = 3           # spatial dims
R = 8           # cores
NL = N // R     # local points per core
KCH = N // 128  # 64 i-chunks
JCH = NL // 128  # 8 local j-chunks
NITER = 5
THETA_GAMMA = 8.0
DEG = 4         # spatial poly degree
M = 35          # monomials for DEG=4 in 3 vars
NGRP = 4        # PE column groups for the main matmul
CP = 32         # padded class dim for the main stationary

C1 = float(2**23) / float(np.log(2.0))
C2A = float(2**30)
GAMMA = 1.0406829  # E[(1+r)/2^r], r~U[0,1): Schraudolph mean ratio

_CACHE = {}


def _build_program():
    import concourse.mybir as mybir
    import concourse.tile as tile
    from concourse import bacc
    from concourse.bass import ts, ds

    f32 = mybir.dt.float32
    bf16 = mybir.dt.bfloat16
    fp8 = mybir.dt.float8e4
    i32 = mybir.dt.int32
    EXP = mybir.ActivationFunctionType.Exp

    nc = bacc.Bacc("TRN2", target_bir_lowering=False, debug=False, num_devices=R)

    # ---- I/O (host-side pre-transposed into row-contiguous layouts) ----
    ffa = nc.dram_tensor("ffa", [42, N], bf16, kind="ExternalInput")
    fla = nc.dram_tensor("fla", [42, NL], bf16, kind="ExternalInput")
    psiTl = nc.dram_tensor("psiTl", [128, JCH, M], bf16, kind="ExternalInput")
    psil = nc.dram_tensor("psil", [M, NL], bf16, kind="ExternalInput")
    pb_init = nc.dram_tensor("pb_init", [128, KCH, CP], fp8, kind="ExternalInput")
    wt0_hi = nc.dram_tensor("wt0_hi", [M, C], bf16, kind="ExternalInput")
    wt0_lo = nc.dram_tensor("wt0_lo", [M, C], bf16, kind="ExternalInput")
    unT_loc = nc.dram_tensor("unT_loc", [128, JCH, C], f32, kind="ExternalInput")
    m2 = nc.dram_tensor("m2", [4 * C, 4 * CP + 4 * C], bf16, kind="ExternalInput")
    idn = nc.dram_tensor("idn", [128, 128], bf16, kind="ExternalInput")
    sel = nc.dram_tensor("sel", [128, C], bf16, kind="ExternalInput")
    qT_out = nc.dram_tensor("qT_out", [128, JCH, C], f32, kind="ExternalOutput")

    b_act = -C2A / C1 + float(np.log(2.0)) + float(np.log(GAMMA))

    with tile.TileContext(nc) as tc:
        with (
            tc.tile_pool(name="const", bufs=1) as const,
            tc.tile_pool(name="state", bufs=1) as state,
            tc.tile_pool(name="spool", bufs=4) as spool,
            tc.tile_pool(name="opool", bufs=2) as opool,
            tc.tile_pool(name="qpool", bufs=4) as qpool,
            tc.tile_pool(name="psY", bufs=4, space="PSUM") as psY,
            tc.tile_pool(name="psP", bufs=1, space="PSUM") as psP,
            tc.tile_pool(name="psS", bufs=1, space="PSUM") as psS,
            tc.tile_pool(name="dram", bufs=4, space="DRAM") as dram,
        ):
            # ---- constants (loads split across engine DMA queues) ----
            ffa_sb = const.tile([42, N], bf16, name="ffa_sb")
            fla_sb = const.tile([42, NL], bf16, name="fla_sb")
            HN = N // 2
            nc.sync.dma_start(fla_sb[:], fla[:])
            nc.sync.dma_start(ffa_sb[0:10, 0:HN], ffa[0:10, 0:HN])
            nc.scalar.dma_start(ffa_sb[32:42, 0:HN], ffa[32:42, 0:HN])
            nc.sync.dma_start(ffa_sb[0:10, HN:N], ffa[0:10, HN:N])
            nc.scalar.dma_start(ffa_sb[32:42, HN:N], ffa[32:42, HN:N])
            psiTl_sb = const.tile([128, JCH, M], bf16, name="psiTl_sb")
            psil_sb = const.tile([M, NL], bf16, name="psil_sb")
            m2_sb = const.tile([4 * C, 4 * CP + 4 * C], bf16, name="m2_sb")
            idn_sb = const.tile([128, 128], bf16, name="idn_sb")
            sel_sb = const.tile([128, C], bf16, name="sel_sb")
            un_sb = const.tile([128, JCH, C], f32, name="un_sb")
            nc.gpsimd.dma_start(psiTl_sb[:], psiTl[:])
            nc.gpsimd.dma_start(psil_sb[:], psil[:])
            nc.gpsimd.dma_start(m2_sb[:], m2[:])
            nc.gpsimd.dma_start(idn_sb[:], idn[:])
            nc.gpsimd.dma_start(sel_sb[:], sel[:])
            nc.gpsimd.dma_start(un_sb[:], unT_loc[:])

            # warm up the collective stack early: pays the first-trigger
            # firmware cost + absorbs launch skew under the slab build
            bi0 = dram.tile([128, 4 * CP + 4 * C], fp8, name="bi0")
            bo0 = dram.tile([R, 128, 4 * CP + 4 * C], fp8, addr_space="Shared", name="bo0")
            z_sb = const.tile([128, 4 * CP + 4 * C], fp8, name="z_sb")
            nc.gpsimd.memset(z_sb[:], 0)
            nc.gpsimd.dma_start(bi0[:], z_sb[:])
            nc.gpsimd.collective_compute(
                "AllGather",
                mybir.AluOpType.bypass,
                replica_groups=[list(range(R))],
                ins=[bi0[:].opt()],
                outs=[bo0[:].opt()],
            )

            # bilateral slab, fp8, SBUF-resident
            slab = const.tile([128, KCH, NL], fp8, name="slab")

            bact_sb = const.tile([128, 1], f32, name="bact_sb")
            nc.gpsimd.memset(bact_sb[:], b_act)

            # premixed bilateral distribution, fp8, zero-padded to CP cols
            pb8_sb = state.tile([128, KCH, CP], fp8, name="pb8_sb")
            nc.sync.dma_start(pb8_sb[:], pb_init[:])
            pb84 = pb8_sb[:].rearrange("p (r y) c -> p r y c", r=R)

            # spatial filter weights (hi/lo bf16 split of f32)
            wt_hi = state.tile([M, C], bf16, name="wt_hi")
            wt_lo = state.tile([M, C], bf16, name="wt_lo")
            nc.scalar.dma_start(wt_hi[:], wt0_hi[:])
            nc.scalar.dma_start(wt_lo[:], wt0_lo[:])
            # gathered W partials (16 = 8 ranks x 2 halves), f32
            wparts_sb = state.tile([M, 2 * R, C], f32, name="wparts_sb")
            w1_sb = state.tile([M, R, C], f32, name="w1_sb")
            w2_sb = state.tile([M, 4, C], f32, name="w2_sb")
            w3_sb = state.tile([M, 2, C], f32, name="w3_sb")
            wf_sb = state.tile([M, 1, C], f32, name="wf_sb")

            # softmax scratch (per half: [128, 4])
            mx_sb = state.tile([128, JCH], f32, name="mx_sb")
            sm_sb = state.tile([128, JCH], f32, name="sm_sb")
            rs_sb = state.tile([128, JCH], f32, name="rs_sb")
            el_sb = state.tile([128, JCH, C], f32, name="el_sb")

            # persistent small-PSUM tiles (sel/qa, premix, W partial, warmers)
            QW = psS.tile([128, 4 * C + C + 16], f32, name="qw")
            PMX = psS.tile([128, 4 * CP + 4 * C], f32, name="pmx")

            def wrm(n):
                # HAM warmers: keep the PE activity monitor hot through
                # dependency stalls so the clock gate stays at 8/8.
                for _ in range(n):
                    nc.tensor.matmul(
                        QW[:, 4 * C + C:4 * C + C + 16],
                        idn_sb[:], idn_sb[:, 0:16],
                        start=True, stop=True,
                    )

            # k-chunk orders: AG-half-0 chunks (y 0..3) then half-1
            ks_h0 = [r * JCH + y for y in range(4) for r in range(R)]
            ks_h1 = [r * JCH + y for y in range(4, JCH) for r in range(R)]
            ks_ag = ks_h0 + ks_h1

            def emit_mains(po, jh, ks):
                """64 accumulation matmuls into po's 4 column bands."""
                seen = set()
                last = {}
                for k in ks:
                    last[k % NGRP] = k
                jsl = ds(jh * 512, 512)
                for k in ks:
                    g = k % NGRP
                    nc.tensor.matmul(
                        po[32 * g:32 * g + CP, :],
                        pb8_sb[:, k, :],
                        slab[:, k, jsl],
                        tile_position=(0, 32 * g),
                        start=(g not in seen),
                        # band 0 is closed later by the osp accumulation
                        stop=(last[g] == k and g != 0),
                    )
                    seen.add(g)

            def emit_osp(po, jh):
                jsl = ds(jh * 512, 512)
                nc.tensor.matmul(
                    po[0:C, :], wt_hi[:], psil_sb[:, jsl],
                    tile_position=(0, 0), start=False, stop=False,
                )
                nc.tensor.matmul(
                    po[0:C, :], wt_lo[:], psil_sb[:, jsl],
                    tile_position=(0, 0), start=False, stop=True,
                )

            def emit_post(po, jh, t):
                jr = ds(4 * jh, 4)
                # PSUM -> SBUF bf16, split across Scalar and Vector
                poS = opool.tile([128, 512], bf16, name="poS")
                nc.scalar.copy(poS[:, 0:256], po[:, 0:256])
                nc.vector.tensor_scalar_add(poS[:, 256:512], po[:, 256:512], 0.0)
                # 4-band sum via sel matmul, straight into [j, c] layout
                qw = QW
                wrm(2)
                for j in range(4):
                    nc.tensor.matmul(
                        qw[:, C * j:C * j + C], poS[:, ts(j, 128)], sel_sb[:],
                        start=True, stop=True,
                    )
                ql = qpool.tile([128, 4, C], f32, name="ql")
                nc.vector.tensor_add(
                    ql[:],
                    qw[:, 0:4 * C].rearrange("p (y c) -> p y c", c=C),
                    un_sb[:, jr, :],
                )
                if t == NITER - 1:
                    nc.sync.dma_start(qT_out[:, jr, :], ql[:])
                    return
                # ---- softmax over classes (free axis) ----
                hsl = ds(4 * jh, 4)
                nc.vector.reduce_max(
                    mx_sb[:, hsl], ql[:], axis=mybir.AxisListType.X
                )
                mx_b = mx_sb[:, hsl].unsqueeze(2).broadcast_to((128, 4, C))
                nc.vector.tensor_sub(el_sb[:, hsl, :], ql[:], mx_b)
                nc.scalar.activation(
                    el_sb[:, hsl, :], el_sb[:, hsl, :], EXP, bias=0.0, scale=1.0,
                )
                nc.vector.reduce_sum(
                    sm_sb[:, hsl], el_sb[:, hsl, :], axis=mybir.AxisListType.X
                )
                nc.vector.reciprocal(rs_sb[:, hsl], sm_sb[:, hsl])
                rs_b = rs_sb[:, hsl].unsqueeze(2).broadcast_to((128, 4, C))
                pl = qpool.tile([128, 4, C], bf16, name="pl")
                nc.vector.tensor_mul(pl[:], el_sb[:, hsl, :], rs_b)
                # ---- premix: pl^T, then block-diag mix -> [pb | ps] ----
                pmx = PMX
                plT = pmx[0:4 * C, 0:128]
                wrm(6)
                nc.tensor.matmul(
                    plT, pl[:].rearrange("p y c -> p (y c)"), idn_sb[:],
                    start=True, stop=True,
                )
                plTs = spool.tile([4 * C, 128], bf16, name="plTs")
                nc.scalar.copy(plTs[:], plT)
                pbps = pmx[:, :]
                nc.tensor.matmul(
                    pbps, plTs[:], m2_sb[:], start=True, stop=True,
                )
                agp = spool.tile([128, 4 * CP + 4 * C], fp8, name="agp")
                nc.scalar.copy(agp[:], pbps)
                # ---- local spatial partial W = Psi_loc @ ps_loc^T ----
                wp = qw[0:M, 4 * C:4 * C + C]
                for j in range(4):
                    nc.tensor.matmul(
                        wp, psiTl_sb[:, 4 * jh + j, :],
                        agp[:, 4 * CP + C * j:4 * CP + C * j + C],
                        start=(j == 0), stop=(j == 3),
                    )
                w_sb = spool.tile([M, C], f32, name="w_sb")
                nc.vector.tensor_scalar_add(w_sb[:], wp, 0.0)
                # ---- bounce + AllGather (pb fp8 cols 0:40, W f32 bytes 40:80) ----
                bi = dram.tile([128, 4 * CP + 4 * C], fp8, name="bi")
                bo = dram.tile([R, 128, 4 * CP + 4 * C], fp8, addr_space="Shared", name="bo")
                eng = nc.sync if jh == 0 else nc.gpsimd
                eng.dma_start(bi[:, 0:4 * CP], agp[:, 0:4 * CP])
                eng.dma_start(
                    bi[0:M, 4 * CP:4 * CP + 4 * C],
                    w_sb[:].bitcast(fp8),
                )
                nc.gpsimd.collective_compute(
                    "AllGather",
                    mybir.AluOpType.bypass,
                    replica_groups=[list(range(R))],
                    ins=[bi[:].opt()],
                    outs=[bo[:].opt()],
                )
                # ---- scatter: pb chunks + W partials ----
                eng.dma_start(
                    pb84[:, :, 4 * jh:4 * jh + 4, :],
                    bo[:, :, 0:4 * CP].rearrange("r p (y c) -> p r y c", c=CP),
                )
                eng.dma_start(
                    wparts_sb[:, R * jh:R * jh + R, :].bitcast(fp8),
                    bo[:, 0:M, 4 * CP:4 * CP + 4 * C].rearrange("r p f -> p r f"),
                )
                if jh == 1:
                    # f32 tree-sum of the 16 W partials -> next iteration's wt
                    nc.vector.tensor_add(
                        w1_sb[:], wparts_sb[:, 0:R, :], wparts_sb[:, R:2 * R, :]
                    )
                    nc.vector.tensor_add(
                        w2_sb[:], w1_sb[:, 0:4, :], w1_sb[:, 4:8, :]
                    )
                    nc.vector.tensor_add(
                        w3_sb[:], w2_sb[:, 0:2, :], w2_sb[:, 2:4, :]
                    )
                    nc.vector.tensor_add(
                        wf_sb[:], w3_sb[:, 0:1, :], w3_sb[:, 1:2, :]
                    )
                    nc.scalar.copy(wt_hi[:], wf_sb[:, 0, :])
                    nc.vector.tensor_sub(wt_lo[:], wf_sb[:, 0, :], wt_hi[:])

            for t in range(NITER):
                po0 = psP.tile([128, 512], f32, name="po0")
                po1 = psP.tile([128, 512], f32, name="po1")
                if t == 0:
                    # slab build interleaved with iteration-0 mains (LAG chunks)
                    LAG = 8
                    seen0, seen1 = set(), set()
                    for kk in range(KCH + LAG):
                        if kk < KCH:
                            k = kk
                            for sh in range(2):
                                # the two halves stream on different PE row
                                # bands so they overlap
                                rs = 32 * ((k + sh) % 2)
                                ssl = ds(sh * 512, 512)
                                yt = psY.tile([128, 512], f32, name="yt")
                                nc.tensor.matmul(
                                    yt[:],
                                    ffa_sb[rs:rs + 10, ts(k, 128)],
                                    fla_sb[rs:rs + 10, ssl],
                                    start=True, stop=True,
                                    tile_position=(rs, 0),
                                )
                                u = 2 * k + sh
                                if u % 8 in (0, 2, 4, 6, 7):
                                    nc.scalar.activation(
                                        slab[:, k, ssl], yt[:], EXP,
                                        bias=bact_sb[:], scale=1.0 / C1,
                                    )
                                else:
                                    sc = spool.tile([128, 512], i32, name="sc")
                                    nc.vector.tensor_scalar_add(
                                        sc[:], yt[:], 0.0
                                    )
                                    nc.vector.tensor_scalar_add(
                                        slab[:, k, ssl], sc[:].bitcast(f32), 0.0
                                    )
                            if k == 40:
                                # second warmup AG: re-syncs the cores mid-slab
                                # so the first data AG doesn't absorb the drift
                                bi0b = dram.tile(
                                    [128, 4 * CP + 4 * C], fp8, name="bi0b"
                                )
                                bo0b = dram.tile(
                                    [R, 128, 4 * CP + 4 * C], fp8,
                                    addr_space="Shared", name="bo0b",
                                )
                                nc.gpsimd.dma_start(bi0b[:], z_sb[:])
                                nc.gpsimd.collective_compute(
                                    "AllGather",
                                    mybir.AluOpType.bypass,
                                    replica_groups=[list(range(R))],
                                    ins=[bi0b[:].opt()],
                                    outs=[bo0b[:].opt()],
                                )
                        if kk >= LAG:
                            k = kk - LAG
                            g = k % NGRP
                            for po, jh, seen in ((po0, 0, seen0), (po1, 1, seen1)):
                                nc.tensor.matmul(
                                    po[32 * g:32 * g + CP, :],
                                    pb8_sb[:, k, :],
                                    slab[:, k, ds(jh * 512, 512)],
                                    tile_position=(0, 32 * g),
                                    start=(g not in seen),
                                    stop=(k >= KCH - NGRP and g != 0),
                                )
                                seen.add(g)
                        wrm(2)
                    emit_osp(po0, 0)
                    emit_osp(po1, 1)
                    emit_post(po0, 0, t)
                    emit_post(po1, 1, t)
                else:
                    wrm(35)
                    seen = set()
                    last = {}
                    for k in ks_ag:
                        last[k % NGRP] = k
                    for k in ks_ag:
                        g = k % NGRP
                        for po, jh in ((po0, 0), (po1, 1)):
                            nc.tensor.matmul(
                                po[32 * g:32 * g + CP, :],
                                pb8_sb[:, k, :],
                                slab[:, k, ds(jh * 512, 512)],
                                tile_position=(0, 32 * g),
                                start=((g, jh) not in seen),
                                stop=(last[g] == k and g != 0),
                            )
                            seen.add((g, jh))
                    emit_osp(po0, 0)
                    emit_osp(po1, 1)
                    emit_post(po0, 0, t)
                    emit_post(po1, 1, t)

    nc.compile()
    return nc


def _get_program():
    if "nc" not in _CACHE:
        _CACHE["nc"] = _build_program()
    return _CACHE["nc"]


def _host_prep(unaries, feat, sw, bw, compat):
    bf = ml_dtypes.bfloat16
    f8 = ml_dtypes.float8_e4m3
    f = feat.astype(np.float32)
    f2 = np.sum(f * f, axis=0)

    sqc = np.float32(np.sqrt(C1))
    fr = (sqc * f).astype(bf)                      # [6, N] bf16 scaled features
    r_row = (np.float32(C1) * (-0.5 * f2)).astype(bf)   # bf16 |f|^2 row

    # exact correction for the bf16 rounding of the j-side row, folded
    # into the exponent as one extra augmented row
    r_used = r_row.astype(np.float32)
    v_row = (r_used + np.float32(C1) * (0.5 * f2).astype(np.float32)).astype(bf)

    # i-side rows (lhsT): [sq*f(6); r_i; 1; 1; 1],
    # j-side rows (rhs):  [sq*f(6); 1; r_j; 2^30; v]
    ffa = np.zeros((42, N), dtype=bf)
    fla_full = np.zeros((42, N), dtype=bf)
    for off in (0, 32):
        ffa[off:off + 6] = fr
        ffa[off + 6] = r_row
        ffa[off + 7] = bf(1.0)
        ffa[off + 8] = bf(1.0)
        ffa[off + 9] = bf(1.0)
        fla_full[off:off + 6] = fr
        fla_full[off + 6] = bf(1.0)
        fla_full[off + 7] = r_row
        fla_full[off + 8] = bf(C2A)
        fla_full[off + 9] = v_row

    # spatial poly features
    from math import factorial
    s = f[:S] / np.float32(THETA_GAMMA)
    a_sp = np.exp(-0.5 * np.sum(s * s, axis=0))
    rows = []
    for a in range(DEG + 1):
        for b in range(DEG + 1 - a):
            for c in range(DEG + 1 - a - b):
                coef = 1.0 / np.sqrt(factorial(a) * factorial(b) * factorial(c))
                rows.append(coef * s[0] ** a * s[1] ** b * s[2] ** c)
    psi = (np.stack(rows) * a_sp[None, :]).astype(bf)    # [M, N]
    # local psi in i-layout per core: [128, JCH, M]
    psiT = np.ascontiguousarray(
        psi.T.reshape(KCH, 128, M).transpose(1, 0, 2)
    )  # [128, KCH, M]

    # premix matrices
    Mbl = -(compat @ bw)
    Msp = -(compat @ sw)
    Mbl_s = (Mbl / np.float32(2.0 * GAMMA)).astype(bf)
    Msp_s = Msp.astype(bf)

    # block-diagonal mix matrix [40, 168]: rows (j,c) ->
    # cols 0:128 = pb (4 x CP-padded blocks), cols 128:168 = ps (4 x C)
    m2 = np.zeros((4 * C, 4 * CP + 4 * C), dtype=bf)
    for j in range(4):
        m2[C * j:C * j + C, CP * j:CP * j + C] = Mbl_s.T
        m2[C * j:C * j + C, 4 * CP + C * j:4 * CP + C * j + C] = Msp_s.T

    idn = np.eye(128, dtype=bf)
    sel = np.zeros((128, C), dtype=bf)
    for g in range(NGRP):
        for c in range(C):
            sel[32 * g + c, c] = bf(1.0)

    # iteration-0 distributions (host softmax + premix)
    mx = unaries.max(axis=0, keepdims=True)
    e = np.exp(unaries - mx, dtype=np.float32)
    p0 = e / e.sum(axis=0, keepdims=True)
    p0 = p0.T.astype(bf).astype(np.float32).T       # device pl is bf16
    pb0 = (Mbl_s.astype(np.float32) @ p0).astype(f8)     # [C, N]
    ps0 = (Msp_s.astype(np.float32) @ p0).astype(f8)
    wt0 = psi.astype(np.float32) @ ps0.astype(np.float32).T   # [M, C] f32
    wt0_hi = wt0.astype(bf)
    wt0_lo = (wt0 - wt0_hi.astype(np.float32)).astype(bf)
    # [128, KCH, CP]: pb0T[p, k, c] = pb0[c, 128k+p], zero-padded
    pb0T = np.zeros((128, KCH, CP), dtype=f8)
    pb0T[:, :, 0:C] = pb0.T.reshape(KCH, 128, C).transpose(1, 0, 2)

    qT_init = np.ascontiguousarray(unaries.T).astype(np.float32)
    return (ffa, fla_full, psiT, psi, m2, idn, sel,
            pb0T, wt0_hi, wt0_lo, qT_init)


def _make_in_maps(inputs):
    unaries = np.asarray(inputs["unaries"], dtype=np.float32)
    feat = np.asarray(inputs["feat"], dtype=np.float32)
    sw = np.asarray(inputs["spatial_weights"], dtype=np.float32)
    bw = np.asarray(inputs["bilateral_weights"], dtype=np.float32)
    compat = np.asarray(inputs["compatibility_matrix"], dtype=np.float32)

    (ffa, fla_full, psiT, psi, m2, idn, sel,
     pb0T, wt0_hi, wt0_lo, qT_init) = _host_prep(unaries, feat, sw, bw, compat)
    in_maps = []
    for r in range(R):
        jsl = slice(r * NL, (r + 1) * NL)
        ksl = slice(r * JCH, (r + 1) * JCH)
        in_maps.append({
            "ffa": ffa,
            "fla": np.ascontiguousarray(fla_full[:, jsl]),
            "psiTl": np.ascontiguousarray(psiT[:, ksl, :]),
            "psil": np.ascontiguousarray(psi[:, jsl]),
            "pb_init": pb0T,
            "wt0_hi": wt0_hi,
            "wt0_lo": wt0_lo,
            "unT_loc": np.ascontiguousarray(
                qT_init[jsl].reshape(JCH, 128, C).transpose(1, 0, 2)
            ),
            "m2": m2,
            "idn": idn,
            "sel": sel,
        })
    return in_maps


def kernel(unaries, feat, spatial_weights, bilateral_weights, compatibility_matrix):
    from concourse.bass_utils import run_bass_kernel_spmd

    in_maps = _make_in_maps({
        "unaries": unaries,
        "feat": feat,
        "spatial_weights": spatial_weights,
        "bilateral_weights": bilateral_weights,
        "compatibility_matrix": compatibility_matrix,
    })
    nc = _get_program()
    res = run_bass_kernel_spmd(nc, in_maps, core_ids=list(range(R)))

    q = np.empty((C, N), dtype=np.float32)
    for r in range(R):
        out = res.results[r]["qT_out"]          # [128, JCH, C]
        q[:, r * NL:(r + 1) * NL] = out.transpose(2, 1, 0).reshape(C, NL)
    return q


# revision 18
# speedup vs baseline: 1.3059x; 1.0546x over previous
"""CRF mean-field (nn_CRF) Trainium2 kernel, SPMD over 8 NeuronCores. v3.

Math: 5 iterations of
    p   = softmax(q, axis=classes)
    q   = unaries - compat @ (sw @ (p @ K_sp) + bw @ (p @ K_bl))

Design (v3, rewritten from v2 to shorten the per-iteration critical path):

  * The CxC mixing commutes with the N-axis filtering, so the mixing is
    PRE-applied to the local p shard before the AllGather:
        pb = (-(compat@bw)/(2*gamma)) @ p   (feeds the bilateral slab matmul)
        ps = (-(compat@sw)) @ p             (feeds the spatial poly path)
    via two tiny PE matmuls (transpose against identity, then a
    block-diagonal [40,80] mix matrix). Only pb is AllGathered; the
    spatial path ships as a per-core partial W_r = Psi_loc @ ps_loc^T
    (f32, byte-packed into the same AG buffer) and the 16 partials are
    tree-summed post-AG. This removes the per-iteration 64-chunk wtp
    accumulation and all post-filter mixing matmuls.

  * Main bilateral matmul: pb8 [128,32] fp8 stationary (zero-padded
    cols 10-31 so every PSUM row is defined) x slab [128,512] fp8
    moving, 4 PE column groups. The 4 band partials + the spatial osp
    (accumulated into band 0) are summed by ONE [128,128]x[128,10]
    sel-matmul per j-chunk after a single PSUM->SBUF bf16 copy --
    replacing v2's serial 2.7us DVE combine chain per half.

  * AllGathers (2/iter, 10KB) overlap compute: the k-loop is ordered
    AG0-chunks then AG1-chunks, and the j-half-1 mains + post run under
    AG0's flight.

  * Slab build unchanged numerically (exp via ScalarE activation and a
    DVE Schraudolph path) but in [128,512] half-tiles with a
    measured-rate 5:3 Scalar:DVE split, and iteration-0's mains are
    interleaved 8 chunks behind the build so they finish with it.
"""

import numpy as np
import ml_dtypes

C = 10          # classes
N = 8192        # points
S = 3           # spatial dims
R = 8           # cores
NL = N // R     # local points per core
KCH = N // 128  # 64 i-chunks
JCH = NL // 128  # 8 local j-chunks
NITER = 5
THETA_GAMMA = 8.0
DEG = 4         # spatial poly degree
M = 35          # monomials for DEG=4 in 3 vars
NGRP = 4        # PE column groups for the main matmul
CP = 32         # padded class dim for the main stationary

C1 = float(2**23) / float(np.log(2.0))
C2A = float(2**30)
GAMMA = 1.0406829  # E[(1+r)/2^r], r~U[0,1): Schraudolph mean ratio

_CACHE = {}


def _build_program():
    import concourse.mybir as mybir
    import concourse.tile as tile
    from concourse import bacc
    from concourse.bass import ts, ds

    f32 = mybir.dt.float32
    bf16 = mybir.dt.bfloat16
    fp8 = mybir.dt.float8e4
    i32 = mybir.dt.int32
    EXP = mybir.ActivationFunctionType.Exp

    nc = bacc.Bacc("TRN2", target_bir_lowering=False, debug=False, num_devices=R)

    # ---- I/O (host-side pre-transposed into row-contiguous layouts) ----
    ffa = nc.dram_tensor("ffa", [42, N], bf16, kind="ExternalInput")
    fla = nc.dram_tensor("fla", [42, NL], bf16, kind="ExternalInput")
    psiTl = nc.dram_tensor("psiTl", [128, JCH, M], bf16, kind="ExternalInput")
    psil = nc.dram_tensor("psil", [M, NL], bf16, kind="ExternalInput")
    pb_init = nc.dram_tensor("pb_init", [128, KCH, CP], fp8, kind="ExternalInput")
    wt0_hi = nc.dram_tensor("wt0_hi", [M, C], bf16, kind="ExternalInput")
    wt0_lo = nc.dram_tensor("wt0_lo", [M, C], bf16, kind="ExternalInput")
    unT_loc = nc.dram_tensor("unT_loc", [128, JCH, C], f32, kind="ExternalInput")
    m2pb = nc.dram_tensor("m2pb", [4 * C, 4 * CP], bf16, kind="ExternalInput")
    m2ps = nc.dram_tensor("m2ps", [4 * C, 4 * C], bf16, kind="ExternalInput")
    idn = nc.dram_tensor("idn", [128, 128], bf16, kind="ExternalInput")
    sel = nc.dram_tensor("sel", [128, C], bf16, kind="ExternalInput")
    qT_out = nc.dram_tensor("qT_out", [128, JCH, C], f32, kind="ExternalOutput")

    b_act = -C2A / C1 + float(np.log(2.0)) + float(np.log(GAMMA))

    with tile.TileContext(nc) as tc:
        with (
            tc.tile_pool(name="const", bufs=1) as const,
            tc.tile_pool(name="state", bufs=1) as state,
            tc.tile_pool(name="spool", bufs=4) as spool,
            tc.tile_pool(name="opool", bufs=2) as opool,
            tc.tile_pool(name="qpool", bufs=4) as qpool,
            tc.tile_pool(name="psY", bufs=4, space="PSUM") as psY,
            tc.tile_pool(name="psP", bufs=1, space="PSUM") as psP,
            tc.tile_pool(name="psS", bufs=1, space="PSUM") as psS,
            tc.tile_pool(name="dram", bufs=4, space="DRAM") as dram,
        ):
            FB = 2 * 4 * CP + JCH * C          # 336: premix output width
            AB = 2 * 4 * CP + 4 * C            # 296: AG payload width
            # ---- constants (loads split across engine DMA queues) ----
            ffa_sb = const.tile([42, N], bf16, name="ffa_sb")
            fla_sb = const.tile([42, NL], bf16, name="fla_sb")
            HN = N // 2
            nc.sync.dma_start(fla_sb[:], fla[:])
            nc.sync.dma_start(ffa_sb[0:10, 0:HN], ffa[0:10, 0:HN])
            nc.scalar.dma_start(ffa_sb[32:42, 0:HN], ffa[32:42, 0:HN])
            nc.sync.dma_start(ffa_sb[0:10, HN:N], ffa[0:10, HN:N])
            nc.scalar.dma_start(ffa_sb[32:42, HN:N], ffa[32:42, HN:N])
            psiTl_sb = const.tile([128, JCH, M], bf16, name="psiTl_sb")
            psil_sb = const.tile([M, NL], bf16, name="psil_sb")
            m2pb_sb = const.tile([4 * C, 4 * CP], bf16, name="m2pb_sb")
            m2ps_sb = const.tile([4 * C, 4 * C], bf16, name="m2ps_sb")
            idn_sb = const.tile([128, 128], bf16, name="idn_sb")
            sel_sb = const.tile([128, C], bf16, name="sel_sb")
            un_sb = const.tile([128, JCH, C], f32, name="un_sb")
            nc.gpsimd.dma_start(psiTl_sb[:], psiTl[:])
            nc.gpsimd.dma_start(psil_sb[:], psil[:])
            nc.gpsimd.dma_start(m2pb_sb[:], m2pb[:])
            nc.gpsimd.dma_start(m2ps_sb[:], m2ps[:])
            nc.gpsimd.dma_start(idn_sb[:], idn[:])
            nc.gpsimd.dma_start(sel_sb[:], sel[:])
            nc.gpsimd.dma_start(un_sb[:], unT_loc[:])

            # warm up the collective stack early: pays the first-trigger
            # firmware cost + absorbs launch skew under the slab build
            bi0 = dram.tile([128, AB], fp8, name="bi0")
            bo0 = dram.tile([R, 128, AB], fp8, addr_space="Shared", name="bo0")
            z_sb = const.tile([128, AB], fp8, name="z_sb")
            nc.gpsimd.memset(z_sb[:], 0)
            nc.gpsimd.dma_start(bi0[:], z_sb[:])
            nc.gpsimd.collective_compute(
                "AllGather",
                mybir.AluOpType.bypass,
                replica_groups=[list(range(R))],
                ins=[bi0[:].opt()],
                outs=[bo0[:].opt()],
            )

            # bilateral slab, fp8, SBUF-resident
            slab = const.tile([128, KCH, NL], fp8, name="slab")

            bact_sb = const.tile([128, 1], f32, name="bact_sb")
            nc.gpsimd.memset(bact_sb[:], b_act)

            # premixed bilateral distribution, fp8, zero-padded to CP cols
            pb8_sb = state.tile([128, KCH, CP], fp8, name="pb8_sb")
            nc.sync.dma_start(pb8_sb[:], pb_init[:])
            pb84 = pb8_sb[:].rearrange("p (r y) c -> p r y c", r=R)

            # spatial filter weights (hi/lo bf16 split of f32)
            wt_hi = state.tile([M, C], bf16, name="wt_hi")
            wt_lo = state.tile([M, C], bf16, name="wt_lo")
            nc.scalar.dma_start(wt_hi[:], wt0_hi[:])
            nc.scalar.dma_start(wt_lo[:], wt0_lo[:])
            # gathered W partials (one per rank), f32
            wparts_sb = state.tile([M, R, C], f32, name="wparts_sb")
            w2_sb = state.tile([M, 4, C], f32, name="w2_sb")
            w3_sb = state.tile([M, 2, C], f32, name="w3_sb")
            wf_sb = state.tile([M, 1, C], f32, name="wf_sb")

            # softmax scratch + local p (bf16)
            mx_sb = state.tile([128, JCH], f32, name="mx_sb")
            sm_sb = state.tile([128, JCH], f32, name="sm_sb")
            rs_sb = state.tile([128, JCH], f32, name="rs_sb")
            el_sb = state.tile([128, JCH, C], f32, name="el_sb")
            pl_sb = state.tile([128, JCH, C], bf16, name="pl_sb")

            # persistent small-PSUM tiles (sel/qa + W + warmers; premix)
            QW = psS.tile([128, JCH * C + C + 16], f32, name="qw")
            PMX = psS.tile([128, FB], f32, name="pmx")

            def wrm(n):
                # skinny HAM warmers for short stalls
                for _ in range(n):
                    nc.tensor.matmul(
                        QW[:, JCH * C + C:JCH * C + C + 16],
                        idn_sb[:], idn_sb[:, 0:16],
                        start=True, stop=True,
                    )

            def wrm_fat(n):
                # fat warmers: full-array 128-col pulses to hold the HAM
                # clock gate at 8/8 through AllGather flight windows
                for _ in range(n):
                    nc.tensor.matmul(
                        PMX[:, 0:128], idn_sb[:], idn_sb[:],
                        start=True, stop=True,
                    )

            def emit_osp(po, jh):
                jsl = ds(jh * 512, 512)
                nc.tensor.matmul(
                    po[0:C, :], wt_hi[:], psil_sb[:, jsl],
                    tile_position=(0, 0), start=False, stop=False,
                )
                nc.tensor.matmul(
                    po[0:C, :], wt_lo[:], psil_sb[:, jsl],
                    tile_position=(0, 0), start=False, stop=True,
                )

            def emit_post_half(po, jh, t):
                jr = ds(4 * jh, 4)
                # PSUM -> SBUF bf16, split across Scalar and Vector
                poS = opool.tile([128, 512], bf16, name="poS")
                nc.scalar.copy(poS[:, 0:256], po[:, 0:256])
                nc.vector.tensor_scalar_add(poS[:, 256:512], po[:, 256:512], 0.0)
                # 4-band sum via sel matmul, straight into [j, c] layout
                wrm(2)
                for j in range(4):
                    jg = 4 * jh + j
                    nc.tensor.matmul(
                        QW[:, C * jg:C * jg + C], poS[:, ts(j, 128)], sel_sb[:],
                        start=True, stop=True,
                    )
                ql = qpool.tile([128, 4, C], f32, name="ql")
                nc.vector.tensor_add(
                    ql[:],
                    QW[:, C * 4 * jh:C * 4 * jh + 4 * C].rearrange(
                        "p (y c) -> p y c", c=C),
                    un_sb[:, jr, :],
                )
                if t == NITER - 1:
                    nc.sync.dma_start(qT_out[:, jr, :], ql[:])
                    return
                # ---- softmax over classes (free axis) ----
                hsl = ds(4 * jh, 4)
                nc.vector.reduce_max(
                    mx_sb[:, hsl], ql[:], axis=mybir.AxisListType.X
                )
                mx_b = mx_sb[:, hsl].unsqueeze(2).broadcast_to((128, 4, C))
                nc.vector.tensor_sub(el_sb[:, hsl, :], ql[:], mx_b)
                nc.scalar.activation(
                    el_sb[:, hsl, :], el_sb[:, hsl, :], EXP, bias=0.0, scale=1.0,
                )
                nc.vector.reduce_sum(
                    sm_sb[:, hsl], el_sb[:, hsl, :], axis=mybir.AxisListType.X
                )
                nc.vector.reciprocal(rs_sb[:, hsl], sm_sb[:, hsl])
                rs_b = rs_sb[:, hsl].unsqueeze(2).broadcast_to((128, 4, C))
                nc.vector.tensor_mul(pl_sb[:, hsl, :], el_sb[:, hsl, :], rs_b)
                # ---- premix: pl^T then block-diag mix into pbps regions ----
                wrm(4)
                plT = PMX[0:4 * C, 128 * jh:128 * jh + 128]
                nc.tensor.matmul(
                    plT, pl_sb[:, hsl, :].rearrange("p y c -> p (y c)"),
                    idn_sb[:], start=True, stop=True,
                )
                plTs = spool.tile([4 * C, 128], bf16, name="plTs")
                nc.scalar.copy(plTs[:], plT)
                nc.tensor.matmul(
                    PMX[:, 128 * CP // 32 * jh:128 * CP // 32 * jh + 4 * CP],
                    plTs[:], m2pb_sb[:], start=True, stop=True,
                )
                nc.tensor.matmul(
                    PMX[:, 2 * 4 * CP + 4 * C * jh:2 * 4 * CP + 4 * C * jh + 4 * C],
                    plTs[:], m2ps_sb[:], start=True, stop=True,
                )

            def emit_ag(t):
                # fp8 copy of the premix output (pb padded | ps)
                agp = spool.tile([128, FB], fp8, name="agp")
                nc.scalar.copy(agp[:, 0:FB // 2], PMX[:, 0:FB // 2])
                nc.vector.tensor_scalar_add(
                    agp[:, FB // 2:FB], PMX[:, FB // 2:FB], 0.0
                )
                # local spatial partial W = Psi_loc @ ps_loc^T
                wp = QW[0:M, JCH * C:JCH * C + C]
                for j in range(JCH):
                    nc.tensor.matmul(
                        wp, psiTl_sb[:, j, :],
                        agp[:, 2 * 4 * CP + C * j:2 * 4 * CP + C * j + C],
                        start=(j == 0), stop=(j == JCH - 1),
                    )
                w_sb = spool.tile([M, C], f32, name="w_sb")
                nc.vector.tensor_scalar_add(w_sb[:], wp, 0.0)
                # bounce + AllGather (pb fp8 cols 0:256, W f32 bytes 256:296)
                bi = dram.tile([128, AB], fp8, name="bi")
                bo = dram.tile([R, 128, AB], fp8, addr_space="Shared", name="bo")
                nc.sync.dma_start(bi[:, 0:2 * 4 * CP], agp[:, 0:2 * 4 * CP])
                nc.gpsimd.dma_start(
                    bi[0:M, 2 * 4 * CP:AB], w_sb[:].bitcast(fp8)
                )
                nc.gpsimd.collective_compute(
                    "AllGather",
                    mybir.AluOpType.bypass,
                    replica_groups=[list(range(R))],
                    ins=[bi[:].opt()],
                    outs=[bo[:].opt()],
                )
                # scatter: pb chunks + W partials
                nc.sync.dma_start(
                    pb84[:, :, :, :],
                    bo[:, :, 0:2 * 4 * CP].rearrange(
                        "r p (y c) -> p r y c", c=CP),
                )
                nc.gpsimd.dma_start(
                    wparts_sb[:].bitcast(fp8),
                    bo[:, 0:M, 2 * 4 * CP:AB].rearrange("r p f -> p r f"),
                )
                # f32 tree-sum of the 8 W partials -> next iteration's wt
                nc.vector.tensor_add(
                    w2_sb[:], wparts_sb[:, 0:4, :], wparts_sb[:, 4:8, :]
                )
                nc.vector.tensor_add(
                    w3_sb[:], w2_sb[:, 0:2, :], w2_sb[:, 2:4, :]
                )
                nc.vector.tensor_add(
                    wf_sb[:], w3_sb[:, 0:1, :], w3_sb[:, 1:2, :]
                )
                nc.scalar.copy(wt_hi[:], wf_sb[:, 0, :])
                nc.vector.tensor_sub(wt_lo[:], wf_sb[:, 0, :], wt_hi[:])

            for t in range(NITER):
                po0 = psP.tile([128, 512], f32, name="po0")
                po1 = psP.tile([128, 512], f32, name="po1")
                if t == 0:
                    # slab build interleaved with iteration-0 mains (LAG chunks)
                    LAG = 8
                    seen = set()
                    for kk in range(KCH + LAG):
                        if kk < KCH:
                            k = kk
                            for sh in range(2):
                                # the two halves stream on different PE row
                                # bands so they overlap
                                rs = 32 * ((k + sh) % 2)
                                ssl = ds(sh * 512, 512)
                                yt = psY.tile([128, 512], f32, name="yt")
                                nc.tensor.matmul(
                                    yt[:],
                                    ffa_sb[rs:rs + 10, ts(k, 128)],
                                    fla_sb[rs:rs + 10, ssl],
                                    start=True, stop=True,
                                    tile_position=(rs, 0),
                                )
                                u = 2 * k + sh
                                if u % 8 in (0, 2, 4, 6, 7):
                                    nc.scalar.activation(
                                        slab[:, k, ssl], yt[:], EXP,
                                        bias=bact_sb[:], scale=1.0 / C1,
                                    )
                                else:
                                    sc = spool.tile([128, 512], i32, name="sc")
                                    nc.vector.tensor_scalar_add(
                                        sc[:], yt[:], 0.0
                                    )
                                    nc.vector.tensor_scalar_add(
                                        slab[:, k, ssl], sc[:].bitcast(f32), 0.0
                                    )
                            if k == 40:
                                # second warmup AG: re-syncs the cores mid-
                                # slab so the first data AG sees no drift
                                bi0b = dram.tile([128, AB], fp8, name="bi0b")
                                bo0b = dram.tile(
                                    [R, 128, AB], fp8,
                                    addr_space="Shared", name="bo0b",
                                )
                                nc.gpsimd.dma_start(bi0b[:], z_sb[:])
                                nc.gpsimd.collective_compute(
                                    "AllGather",
                                    mybir.AluOpType.bypass,
                                    replica_groups=[list(range(R))],
                                    ins=[bi0b[:].opt()],
                                    outs=[bo0b[:].opt()],
                                )
                        if kk >= LAG:
                            k = kk - LAG
                            g = k % NGRP
                            for po, jh in ((po0, 0), (po1, 1)):
                                nc.tensor.matmul(
                                    po[32 * g:32 * g + CP, :],
                                    pb8_sb[:, k, :],
                                    slab[:, k, ds(jh * 512, 512)],
                                    tile_position=(0, 32 * g),
                                    start=((g, jh) not in seen),
                                    stop=(k >= KCH - NGRP and g != 0),
                                )
                                seen.add((g, jh))
                else:
                    wrm_fat(26)
                    seen = set()
                    for k in range(KCH):
                        g = k % NGRP
                        for po, jh in ((po0, 0), (po1, 1)):
                            nc.tensor.matmul(
                                po[32 * g:32 * g + CP, :],
                                pb8_sb[:, k, :],
                                slab[:, k, ds(jh * 512, 512)],
                                tile_position=(0, 32 * g),
                                start=((g, jh) not in seen),
                                stop=(k >= KCH - NGRP and g != 0),
                            )
                            seen.add((g, jh))
                emit_osp(po0, 0)
                emit_osp(po1, 1)
                emit_post_half(po0, 0, t)
                emit_post_half(po1, 1, t)
                if t < NITER - 1:
                    emit_ag(t)

    nc.compile()
    return nc


def _get_program():
    if "nc" not in _CACHE:
        _CACHE["nc"] = _build_program()
    return _CACHE["nc"]


def _host_prep(unaries, feat, sw, bw, compat):
    bf = ml_dtypes.bfloat16
    f8 = ml_dtypes.float8_e4m3
    f = feat.astype(np.float32)
    f2 = np.sum(f * f, axis=0)

    sqc = np.float32(np.sqrt(C1))
    fr = (sqc * f).astype(bf)                      # [6, N] bf16 scaled features
    r_row = (np.float32(C1) * (-0.5 * f2)).astype(bf)   # bf16 |f|^2 row

    # exact correction for the bf16 rounding of the j-side row, folded
    # into the exponent as one extra augmented row
    r_used = r_row.astype(np.float32)
    v_row = (r_used + np.float32(C1) * (0.5 * f2).astype(np.float32)).astype(bf)

    # i-side rows (lhsT): [sq*f(6); r_i; 1; 1; 1],
    # j-side rows (rhs):  [sq*f(6); 1; r_j; 2^30; v]
    ffa = np.zeros((42, N), dtype=bf)
    fla_full = np.zeros((42, N), dtype=bf)
    for off in (0, 32):
        ffa[off:off + 6] = fr
        ffa[off + 6] = r_row
        ffa[off + 7] = bf(1.0)
        ffa[off + 8] = bf(1.0)
        ffa[off + 9] = bf(1.0)
        fla_full[off:off + 6] = fr
        fla_full[off + 6] = bf(1.0)
        fla_full[off + 7] = r_row
        fla_full[off + 8] = bf(C2A)
        fla_full[off + 9] = v_row

    # spatial poly features
    from math import factorial
    s = f[:S] / np.float32(THETA_GAMMA)
    a_sp = np.exp(-0.5 * np.sum(s * s, axis=0))
    rows = []
    for a in range(DEG + 1):
        for b in range(DEG + 1 - a):
            for c in range(DEG + 1 - a - b):
                coef = 1.0 / np.sqrt(factorial(a) * factorial(b) * factorial(c))
                rows.append(coef * s[0] ** a * s[1] ** b * s[2] ** c)
    psi = (np.stack(rows) * a_sp[None, :]).astype(bf)    # [M, N]
    # local psi in i-layout per core: [128, JCH, M]
    psiT = np.ascontiguousarray(
        psi.T.reshape(KCH, 128, M).transpose(1, 0, 2)
    )  # [128, KCH, M]

    # premix matrices
    Mbl = -(compat @ bw)
    Msp = -(compat @ sw)
    Mbl_s = (Mbl / np.float32(2.0 * GAMMA)).astype(bf)
    Msp_s = Msp.astype(bf)

    # per-half block-diagonal mix matrices: rows (j,c) ->
    # m2pb [40, 128]: 4 CP-padded pb blocks; m2ps [40, 40]: 4 ps blocks
    m2pb = np.zeros((4 * C, 4 * CP), dtype=bf)
    m2ps = np.zeros((4 * C, 4 * C), dtype=bf)
    for j in range(4):
        m2pb[C * j:C * j + C, CP * j:CP * j + C] = Mbl_s.T
        m2ps[C * j:C * j + C, C * j:C * j + C] = Msp_s.T

    idn = np.eye(128, dtype=bf)
    sel = np.zeros((128, C), dtype=bf)
    for g in range(NGRP):
        for c in range(C):
            sel[32 * g + c, c] = bf(1.0)

    # iteration-0 distributions (host softmax + premix)
    mx = unaries.max(axis=0, keepdims=True)
    e = np.exp(unaries - mx, dtype=np.float32)
    p0 = e / e.sum(axis=0, keepdims=True)
    p0 = p0.T.astype(bf).astype(np.float32).T       # device pl is bf16
    pb0 = (Mbl_s.astype(np.float32) @ p0).astype(f8)     # [C, N]
    ps0 = (Msp_s.astype(np.float32) @ p0).astype(f8)
    wt0 = psi.astype(np.float32) @ ps0.astype(np.float32).T   # [M, C] f32
    wt0_hi = wt0.astype(bf)
    wt0_lo = (wt0 - wt0_hi.astype(np.float32)).astype(bf)
    # [128, KCH, CP]: pb0T[p, k, c] = pb0[c, 128k+p], zero-padded
    pb0T = np.zeros((128, KCH, CP), dtype=f8)
    pb0T[:, :, 0:C] = pb0.T.reshape(KCH, 128, C).transpose(1, 0, 2)

    qT_init = np.ascontiguousarray(unaries.T).astype(np.float32)
    return (ffa, fla_full, psiT, psi, m2pb, m2ps, idn, sel,
            pb0T, wt0_hi, wt0_lo, qT_init)


def _make_in_maps(inputs):
    unaries = np.asarray(inputs["unaries"], dtype=np.float32)
    feat = np.asarray(inputs["feat"], dtype=np.float32)
    sw = np.asarray(inputs["spatial_weights"], dtype=np.float32)
    bw = np.asarray(inputs["bilateral_weights"], dtype=np.float32)
    compat = np.asarray(inputs["compatibility_matrix"], dtype=np.float32)

    (ffa, fla_full, psiT, psi, m2pb, m2ps, idn, sel,
     pb0T, wt0_hi, wt0_lo, qT_init) = _host_prep(unaries, feat, sw, bw, compat)
    in_maps = []
    for r in range(R):
        jsl = slice(r * NL, (r + 1) * NL)
        ksl = slice(r * JCH, (r + 1) * JCH)
        in_maps.append({
            "ffa": ffa,
            "fla": np.ascontiguousarray(fla_full[:, jsl]),
            "psiTl": np.ascontiguousarray(psiT[:, ksl, :]),
            "psil": np.ascontiguousarray(psi[:, jsl]),
            "pb_init": pb0T,
            "wt0_hi": wt0_hi,
            "wt0_lo": wt0_lo,
            "unT_loc": np.ascontiguousarray(
                qT_init[jsl].reshape(JCH, 128, C).transpose(1, 0, 2)
            ),
            "m2pb": m2pb,
            "m2ps": m2ps,
            "idn": idn,
            "sel": sel,
        })
    return in_maps


def kernel(unaries, feat, spatial_weights, bilateral_weights, compatibility_matrix):
    from concourse.bass_utils import run_bass_kernel_spmd

    in_maps = _make_in_maps({
        "unaries": unaries,
        "feat": feat,
        "spatial_weights": spatial_weights,
        "bilateral_weights": bilateral_weights,
        "compatibility_matrix": compatibility_matrix,
    })
    nc = _get_program()
    res = run_bass_kernel_spmd(nc, in_maps, core_ids=list(range(R)))

    q = np.empty((C, N), dtype=np.float32)
    for r in range(R):
        out = res.results[r]["qT_out"]          # [128, JCH, C]
        q[:, r * NL:(r + 1) * NL] = out.transpose(2, 1, 0).reshape(C, NL)
    return q


# revision 20
# speedup vs baseline: 1.5733x; 1.2048x over previous
"""CRF mean-field (nn_CRF) Trainium2 kernel, SPMD over 8 NeuronCores. v3.

Math: 5 iterations of
    p   = softmax(q, axis=classes)
    q   = unaries - compat @ (sw @ (p @ K_sp) + bw @ (p @ K_bl))

Design (v3, rewritten from v2 to shorten the per-iteration critical path):

  * The CxC mixing commutes with the N-axis filtering, so the mixing is
    PRE-applied to the local p shard before the AllGather:
        pb = (-(compat@bw)/(2*gamma)) @ p   (feeds the bilateral slab matmul)
        ps = (-(compat@sw)) @ p             (feeds the spatial poly path)
    via two tiny PE matmuls (transpose against identity, then a
    block-diagonal [40,80] mix matrix). Only pb is AllGathered; the
    spatial path ships as a per-core partial W_r = Psi_loc @ ps_loc^T
    (f32, byte-packed into the same AG buffer) and the 16 partials are
    tree-summed post-AG. This removes the per-iteration 64-chunk wtp
    accumulation and all post-filter mixing matmuls.

  * Main bilateral matmul: pb8 [128,32] fp8 stationary (zero-padded
    cols 10-31 so every PSUM row is defined) x slab [128,512] fp8
    moving, 4 PE column groups. The 4 band partials + the spatial osp
    (accumulated into band 0) are summed by ONE [128,128]x[128,10]
    sel-matmul per j-chunk after a single PSUM->SBUF bf16 copy --
    replacing v2's serial 2.7us DVE combine chain per half.

  * AllGathers (2/iter, 10KB) overlap compute: the k-loop is ordered
    AG0-chunks then AG1-chunks, and the j-half-1 mains + post run under
    AG0's flight.

  * Slab build unchanged numerically (exp via ScalarE activation and a
    DVE Schraudolph path) but in [128,512] half-tiles with a
    measured-rate 5:3 Scalar:DVE split, and iteration-0's mains are
    interleaved 8 chunks behind the build so they finish with it.
"""

import numpy as np
import ml_dtypes

C = 10          # classes
N = 8192        # points
S = 3           # spatial dims
R = 8           # cores
NL = N // R     # local points per core
KCH = N // 128  # 64 i-chunks
JCH = NL // 128  # 8 local j-chunks
NITER = 5
THETA_GAMMA = 8.0
DEG = 4         # spatial poly degree
M = 35          # monomials for DEG=4 in 3 vars
NGRP = 4        # PE column groups for the main matmul
CP = 32         # padded class dim for the main stationary

C1 = float(2**23) / float(np.log(2.0))
C2A = float(2**30)
GAMMA = 1.0406829  # E[(1+r)/2^r], r~U[0,1): Schraudolph mean ratio

_CACHE = {}


def _build_program():
    import concourse.mybir as mybir
    import concourse.tile as tile
    from concourse import bacc
    from concourse.bass import ts, ds

    f32 = mybir.dt.float32
    bf16 = mybir.dt.bfloat16
    fp8 = mybir.dt.float8e4
    i32 = mybir.dt.int32
    EXP = mybir.ActivationFunctionType.Exp

    nc = bacc.Bacc("TRN2", target_bir_lowering=False, debug=False, num_devices=R)

    # ---- I/O (host-side pre-transposed into row-contiguous layouts) ----
    ffa = nc.dram_tensor("ffa", [128, N], bf16, kind="ExternalInput")
    fla = nc.dram_tensor("fla", [128, NL], bf16, kind="ExternalInput")
    psiTl = nc.dram_tensor("psiTl", [128, JCH, M], bf16, kind="ExternalInput")
    psil = nc.dram_tensor("psil", [M, NL], bf16, kind="ExternalInput")
    pb_init = nc.dram_tensor("pb_init", [128, KCH, CP], fp8, kind="ExternalInput")
    wt0_hi = nc.dram_tensor("wt0_hi", [M, C], bf16, kind="ExternalInput")
    wt0_lo = nc.dram_tensor("wt0_lo", [M, C], bf16, kind="ExternalInput")
    unT_loc = nc.dram_tensor("unT_loc", [128, JCH, C], f32, kind="ExternalInput")
    m2pb = nc.dram_tensor("m2pb", [4 * C, 4 * CP], bf16, kind="ExternalInput")
    m2ps = nc.dram_tensor("m2ps", [4 * C, 4 * C], bf16, kind="ExternalInput")
    idn = nc.dram_tensor("idn", [128, 128], bf16, kind="ExternalInput")
    sel = nc.dram_tensor("sel", [128, C], bf16, kind="ExternalInput")
    qT_out = nc.dram_tensor("qT_out", [128, JCH, C], f32, kind="ExternalOutput")

    b_act = -C2A / C1 + float(np.log(2.0)) + float(np.log(GAMMA))

    with tile.TileContext(nc) as tc:
        with (
            tc.tile_pool(name="const", bufs=1) as const,
            tc.tile_pool(name="state", bufs=1) as state,
            tc.tile_pool(name="spool", bufs=4) as spool,
            tc.tile_pool(name="opool", bufs=2) as opool,
            tc.tile_pool(name="qpool", bufs=4) as qpool,
            tc.tile_pool(name="psY", bufs=4, space="PSUM") as psY,
            tc.tile_pool(name="psP", bufs=1, space="PSUM") as psP,
            tc.tile_pool(name="psS", bufs=1, space="PSUM") as psS,
            tc.tile_pool(name="dram", bufs=4, space="DRAM") as dram,
        ):
            FB = 2 * 4 * CP + JCH * C          # 336: premix output width
            AB = 2 * 4 * CP + 4 * C            # 296: AG payload width
            # ---- constants (loads split across engine DMA queues) ----
            ffa_sb = const.tile([128, N], bf16, name="ffa_sb")
            fla_sb = const.tile([128, NL], bf16, name="fla_sb")
            HN = N // 2
            nc.sync.dma_start(fla_sb[:], fla[:])
            for off in (0, 32, 64, 96):
                eng = nc.sync if off in (0, 64) else nc.scalar
                eng.dma_start(
                    ffa_sb[off:off + 10, 0:HN], ffa[off:off + 10, 0:HN]
                )
                eng.dma_start(
                    ffa_sb[off:off + 10, HN:N], ffa[off:off + 10, HN:N]
                )
            psiTl_sb = const.tile([128, JCH, M], bf16, name="psiTl_sb")
            psil_sb = const.tile([M, NL], bf16, name="psil_sb")
            m2pb_sb = const.tile([4 * C, 4 * CP], bf16, name="m2pb_sb")
            m2ps_sb = const.tile([4 * C, 4 * C], bf16, name="m2ps_sb")
            idn_sb = const.tile([128, 128], bf16, name="idn_sb")
            sel_sb = const.tile([128, C], bf16, name="sel_sb")
            un_sb = const.tile([128, JCH, C], f32, name="un_sb")
            nc.gpsimd.dma_start(psiTl_sb[:], psiTl[:])
            nc.gpsimd.dma_start(psil_sb[:], psil[:])
            nc.gpsimd.dma_start(m2pb_sb[:], m2pb[:])
            nc.gpsimd.dma_start(m2ps_sb[:], m2ps[:])
            nc.gpsimd.dma_start(idn_sb[:], idn[:])
            nc.gpsimd.dma_start(sel_sb[:], sel[:])
            nc.gpsimd.dma_start(un_sb[:], unT_loc[:])

            # warm up the collective stack early: pays the first-trigger
            # firmware cost + absorbs launch skew under the slab build
            bi0 = dram.tile([128, AB], fp8, name="bi0")
            bo0 = dram.tile([R, 128, AB], fp8, addr_space="Shared", name="bo0")
            z_sb = const.tile([128, AB], fp8, name="z_sb")
            nc.gpsimd.memset(z_sb[:], 0)
            nc.gpsimd.dma_start(bi0[:], z_sb[:])
            nc.gpsimd.collective_compute(
                "AllGather",
                mybir.AluOpType.bypass,
                replica_groups=[list(range(R))],
                ins=[bi0[:].opt()],
                outs=[bo0[:].opt()],
            )

            # bilateral slab, fp8, SBUF-resident
            slab = const.tile([128, KCH, NL], fp8, name="slab")

            bact_sb = const.tile([128, 1], f32, name="bact_sb")
            nc.gpsimd.memset(bact_sb[:], b_act)

            # premixed bilateral distribution, fp8, zero-padded to CP cols
            pb8_sb = state.tile([128, KCH, CP], fp8, name="pb8_sb")
            nc.sync.dma_start(pb8_sb[:], pb_init[:])
            pb84 = pb8_sb[:].rearrange("p (r y) c -> p r y c", r=R)

            # spatial filter weights (hi/lo bf16 split of f32)
            wt_hi = state.tile([M, C], bf16, name="wt_hi")
            wt_lo = state.tile([M, C], bf16, name="wt_lo")
            nc.scalar.dma_start(wt_hi[:], wt0_hi[:])
            nc.scalar.dma_start(wt_lo[:], wt0_lo[:])
            # gathered W partials (one per rank), f32
            wparts_sb = state.tile([M, R, C], f32, name="wparts_sb")
            w2_sb = state.tile([M, 4, C], f32, name="w2_sb")
            w3_sb = state.tile([M, 2, C], f32, name="w3_sb")
            wf_sb = state.tile([M, 1, C], f32, name="wf_sb")

            # softmax scratch + local p (bf16)
            mx_sb = state.tile([128, JCH], f32, name="mx_sb")
            sm_sb = state.tile([128, JCH], f32, name="sm_sb")
            rs_sb = state.tile([128, JCH], f32, name="rs_sb")
            el_sb = state.tile([128, JCH, C], f32, name="el_sb")
            pl_sb = state.tile([128, JCH, C], bf16, name="pl_sb")

            # persistent small-PSUM tiles (sel/qa + W + warmers; premix)
            QW = psS.tile([128, JCH * C + C + 16], f32, name="qw")
            PMX = psS.tile([128, FB], f32, name="pmx")

            def wrm(n):
                # skinny HAM warmers for short stalls
                for _ in range(n):
                    nc.tensor.matmul(
                        QW[:, JCH * C + C:JCH * C + C + 16],
                        idn_sb[:], idn_sb[:, 0:16],
                        start=True, stop=True,
                    )

            def wrm_fat(n):
                # fat warmers: full-array 128-col pulses to hold the HAM
                # clock gate at 8/8 through AllGather flight windows
                for _ in range(n):
                    nc.tensor.matmul(
                        PMX[:, 0:128], idn_sb[:], idn_sb[:],
                        start=True, stop=True,
                    )

            def emit_osp(po, jh):
                jsl = ds(jh * 512, 512)
                nc.tensor.matmul(
                    po[0:C, :], wt_hi[:], psil_sb[:, jsl],
                    tile_position=(0, 0), start=False, stop=False,
                )
                nc.tensor.matmul(
                    po[0:C, :], wt_lo[:], psil_sb[:, jsl],
                    tile_position=(0, 0), start=False, stop=True,
                )

            def emit_poS(po, jh):
                # PSUM -> SBUF bf16, split across Scalar and Vector
                poS = opool.tile([128, 512], bf16, name="poS")
                nc.scalar.copy(poS[:, 0:256], po[:, 0:256])
                nc.vector.tensor_scalar_add(poS[:, 256:512], po[:, 256:512], 0.0)
                return poS

            def emit_sel(poS, jh):
                # 4-band sum via sel matmul, straight into [j, c] layout
                for j in range(4):
                    jg = 4 * jh + j
                    nc.tensor.matmul(
                        QW[:, C * jg:C * jg + C], poS[:, ts(j, 128)], sel_sb[:],
                        start=True, stop=True,
                    )

            def emit_softmax(jh, t):
                jr = ds(4 * jh, 4)
                ql = qpool.tile([128, 4, C], f32, name="ql")
                nc.vector.tensor_add(
                    ql[:],
                    QW[:, C * 4 * jh:C * 4 * jh + 4 * C].rearrange(
                        "p (y c) -> p y c", c=C),
                    un_sb[:, jr, :],
                )
                if t == NITER - 1:
                    nc.sync.dma_start(qT_out[:, jr, :], ql[:])
                    return
                hsl = ds(4 * jh, 4)
                nc.vector.reduce_max(
                    mx_sb[:, hsl], ql[:], axis=mybir.AxisListType.X
                )
                mx_b = mx_sb[:, hsl].unsqueeze(2).broadcast_to((128, 4, C))
                nc.vector.tensor_sub(el_sb[:, hsl, :], ql[:], mx_b)
                nc.scalar.activation(
                    el_sb[:, hsl, :], el_sb[:, hsl, :], EXP, bias=0.0, scale=1.0,
                )
                nc.vector.reduce_sum(
                    sm_sb[:, hsl], el_sb[:, hsl, :], axis=mybir.AxisListType.X
                )
                nc.vector.reciprocal(rs_sb[:, hsl], sm_sb[:, hsl])
                rs_b = rs_sb[:, hsl].unsqueeze(2).broadcast_to((128, 4, C))
                nc.vector.tensor_mul(pl_sb[:, hsl, :], el_sb[:, hsl, :], rs_b)

            def emit_premix(jh):
                # pl^T against identity, then block-diag mix -> pbps regions
                hsl = ds(4 * jh, 4)
                plT = PMX[0:4 * C, 128 * jh:128 * jh + 128]
                nc.tensor.matmul(
                    plT, pl_sb[:, hsl, :].rearrange("p y c -> p (y c)"),
                    idn_sb[:], start=True, stop=True,
                )
                plTs = spool.tile([4 * C, 128], bf16, name="plTs")
                nc.scalar.copy(plTs[:], plT)
                nc.tensor.matmul(
                    PMX[:, 4 * CP * jh:4 * CP * jh + 4 * CP],
                    plTs[:], m2pb_sb[:], start=True, stop=True,
                )
                nc.tensor.matmul(
                    PMX[:, 2 * 4 * CP + 4 * C * jh:2 * 4 * CP + 4 * C * jh + 4 * C],
                    plTs[:], m2ps_sb[:], start=True, stop=True,
                )

            def emit_ag(t):
                # fp8 copy of the premix output: ps part first so the W
                # matmuls can start while the pb parts copy
                agp = spool.tile([128, FB], fp8, name="agp")
                nc.scalar.copy(
                    agp[:, 2 * 4 * CP:FB], PMX[:, 2 * 4 * CP:FB]
                )
                nc.scalar.copy(agp[:, 0:4 * CP], PMX[:, 0:4 * CP])
                nc.vector.tensor_scalar_add(
                    agp[:, 4 * CP:2 * 4 * CP], PMX[:, 4 * CP:2 * 4 * CP], 0.0
                )
                # local spatial partial W = Psi_loc @ ps_loc^T
                wp = QW[0:M, JCH * C:JCH * C + C]
                for j in range(JCH):
                    nc.tensor.matmul(
                        wp, psiTl_sb[:, j, :],
                        agp[:, 2 * 4 * CP + C * j:2 * 4 * CP + C * j + C],
                        start=(j == 0), stop=(j == JCH - 1),
                    )
                w_sb = spool.tile([M, C], f32, name="w_sb")
                nc.vector.tensor_scalar_add(w_sb[:], wp, 0.0)
                # bounce + AllGather (pb fp8 cols 0:256, W f32 bytes 256:296)
                bi = dram.tile([128, AB], fp8, name="bi")
                bo = dram.tile([R, 128, AB], fp8, addr_space="Shared", name="bo")
                nc.sync.dma_start(bi[:, 0:2 * 4 * CP], agp[:, 0:2 * 4 * CP])
                nc.gpsimd.dma_start(
                    bi[0:M, 2 * 4 * CP:AB], w_sb[:].bitcast(fp8)
                )
                nc.gpsimd.collective_compute(
                    "AllGather",
                    mybir.AluOpType.bypass,
                    replica_groups=[list(range(R))],
                    ins=[bi[:].opt()],
                    outs=[bo[:].opt()],
                )
                # scatter: pb chunks (split across queues) + W partials
                nc.sync.dma_start(
                    pb84[:, 0:4, :, :],
                    bo[0:4, :, 0:2 * 4 * CP].rearrange(
                        "r p (y c) -> p r y c", c=CP),
                )
                nc.gpsimd.dma_start(
                    pb84[:, 4:8, :, :],
                    bo[4:8, :, 0:2 * 4 * CP].rearrange(
                        "r p (y c) -> p r y c", c=CP),
                )
                nc.gpsimd.dma_start(
                    wparts_sb[:].bitcast(fp8),
                    bo[:, 0:M, 2 * 4 * CP:AB].rearrange("r p f -> p r f"),
                )
                # f32 tree-sum of the 8 W partials -> next iteration's wt
                nc.vector.tensor_add(
                    w2_sb[:], wparts_sb[:, 0:4, :], wparts_sb[:, 4:8, :]
                )
                nc.vector.tensor_add(
                    w3_sb[:], w2_sb[:, 0:2, :], w2_sb[:, 2:4, :]
                )
                nc.vector.tensor_add(
                    wf_sb[:], w3_sb[:, 0:1, :], w3_sb[:, 1:2, :]
                )
                nc.scalar.copy(wt_hi[:], wf_sb[:, 0, :])
                nc.vector.tensor_sub(wt_lo[:], wf_sb[:, 0, :], wt_hi[:])

            def emit_main(po, jh, k, seen):
                g = k % NGRP
                nc.tensor.matmul(
                    po[32 * g:32 * g + CP, :],
                    pb8_sb[:, k, :],
                    slab[:, k, ds(jh * 512, 512)],
                    tile_position=(0, 32 * g),
                    start=((g, jh) not in seen),
                    stop=(k >= KCH - NGRP and g != 0),
                )
                seen.add((g, jh))

            for t in range(NITER):
                po0 = psP.tile([128, 512], f32, name="po0")
                po1 = psP.tile([128, 512], f32, name="po1")
                seen = set()
                if t == 0:
                    # slab build interleaved with iteration-0 mains (LAG chunks)
                    LAG = 8
                    for kk in range(KCH + LAG):
                        if kk < KCH:
                            k = kk
                            for sh in range(2):
                                # rotate across all four PE row bands
                                rs = 32 * ((2 * k + sh) % 4)
                                ssl = ds(sh * 512, 512)
                                yt = psY.tile([128, 512], f32, name="yt")
                                nc.tensor.matmul(
                                    yt[:],
                                    ffa_sb[rs:rs + 10, ts(k, 128)],
                                    fla_sb[rs:rs + 10, ssl],
                                    start=True, stop=True,
                                    tile_position=(rs, 0),
                                )
                                u = 2 * k + sh
                                if u % 8 in (0, 2, 4, 6, 7):
                                    nc.scalar.activation(
                                        slab[:, k, ssl], yt[:], EXP,
                                        bias=bact_sb[:], scale=1.0 / C1,
                                    )
                                else:
                                    sc = spool.tile([128, 512], i32, name="sc")
                                    nc.vector.tensor_scalar_add(
                                        sc[:], yt[:], 0.0
                                    )
                                    nc.vector.tensor_scalar_add(
                                        slab[:, k, ssl], sc[:].bitcast(f32), 0.0
                                    )
                            if k == 40:
                                # second warmup AG, gated on slab progress:
                                # re-syncs the cores mid-slab so the first
                                # data AG sees little drift
                                bi0b = dram.tile([128, AB], fp8, name="bi0b")
                                bo0b = dram.tile(
                                    [R, 128, AB], fp8,
                                    addr_space="Shared", name="bo0b",
                                )
                                nc.gpsimd.dma_start(
                                    bi0b[:], slab[:, 40, 0:AB]
                                )
                                nc.gpsimd.collective_compute(
                                    "AllGather",
                                    mybir.AluOpType.bypass,
                                    replica_groups=[list(range(R))],
                                    ins=[bi0b[:].opt()],
                                    outs=[bo0b[:].opt()],
                                )
                        if kk >= LAG:
                            k = kk - LAG
                            emit_main(po0, 0, k, seen)
                            emit_main(po1, 1, k, seen)
                    emit_osp(po0, 0)
                    emit_osp(po1, 1)
                    poS0 = emit_poS(po0, 0)
                    emit_sel(poS0, 0)
                else:
                    wrm_fat(48)
                    for k in range(KCH):
                        emit_main(po0, 0, k, seen)
                    emit_osp(po0, 0)
                    poS0 = emit_poS(po0, 0)
                    for k in range(12):
                        emit_main(po1, 1, k, seen)
                    emit_sel(poS0, 0)
                    for k in range(12, KCH):
                        emit_main(po1, 1, k, seen)
                    emit_osp(po1, 1)
                # ---- tail: softmax h0 overlaps po1 drain; premixes chain ----
                emit_softmax(0, t)
                poS1 = emit_poS(po1, 1)
                wrm(2)
                emit_sel(poS1, 1)
                if t < NITER - 1:
                    emit_premix(0)
                    emit_softmax(1, t)
                    emit_premix(1)
                    emit_ag(t)
                else:
                    emit_softmax(1, t)

    nc.compile()
    return nc


def _get_program():
    if "nc" not in _CACHE:
        _CACHE["nc"] = _build_program()
    return _CACHE["nc"]


def _host_prep(unaries, feat, sw, bw, compat):
    bf = ml_dtypes.bfloat16
    f8 = ml_dtypes.float8_e4m3
    f = feat.astype(np.float32)
    f2 = np.sum(f * f, axis=0)

    sqc = np.float32(np.sqrt(C1))
    fr = (sqc * f).astype(bf)                      # [6, N] bf16 scaled features
    r_row = (np.float32(C1) * (-0.5 * f2)).astype(bf)   # bf16 |f|^2 row

    # exact correction for the bf16 rounding of the j-side row, folded
    # into the exponent as one extra augmented row
    r_used = r_row.astype(np.float32)
    v_row = (r_used + np.float32(C1) * (0.5 * f2).astype(np.float32)).astype(bf)

    # i-side rows (lhsT): [sq*f(6); r_i; 1; 1; 1],
    # j-side rows (rhs):  [sq*f(6); 1; r_j; 2^30; v]
    ffa = np.zeros((128, N), dtype=bf)
    fla_full = np.zeros((128, N), dtype=bf)
    for off in (0, 32, 64, 96):
        ffa[off:off + 6] = fr
        ffa[off + 6] = r_row
        ffa[off + 7] = bf(1.0)
        ffa[off + 8] = bf(1.0)
        ffa[off + 9] = bf(1.0)
        fla_full[off:off + 6] = fr
        fla_full[off + 6] = bf(1.0)
        fla_full[off + 7] = r_row
        fla_full[off + 8] = bf(C2A)
        fla_full[off + 9] = v_row

    # spatial poly features
    from math import factorial
    s = f[:S] / np.float32(THETA_GAMMA)
    a_sp = np.exp(-0.5 * np.sum(s * s, axis=0))
    rows = []
    for a in range(DEG + 1):
        for b in range(DEG + 1 - a):
            for c in range(DEG + 1 - a - b):
                coef = 1.0 / np.sqrt(factorial(a) * factorial(b) * factorial(c))
                rows.append(coef * s[0] ** a * s[1] ** b * s[2] ** c)
    psi = (np.stack(rows) * a_sp[None, :]).astype(bf)    # [M, N]
    # local psi in i-layout per core: [128, JCH, M]
    psiT = np.ascontiguousarray(
        psi.T.reshape(KCH, 128, M).transpose(1, 0, 2)
    )  # [128, KCH, M]

    # premix matrices
    Mbl = -(compat @ bw)
    Msp = -(compat @ sw)
    Mbl_s = (Mbl / np.float32(2.0 * GAMMA)).astype(bf)
    Msp_s = Msp.astype(bf)

    # per-half block-diagonal mix matrices: rows (j,c) ->
    # m2pb [40, 128]: 4 CP-padded pb blocks; m2ps [40, 40]: 4 ps blocks
    m2pb = np.zeros((4 * C, 4 * CP), dtype=bf)
    m2ps = np.zeros((4 * C, 4 * C), dtype=bf)
    for j in range(4):
        m2pb[C * j:C * j + C, CP * j:CP * j + C] = Mbl_s.T
        m2ps[C * j:C * j + C, C * j:C * j + C] = Msp_s.T

    idn = np.eye(128, dtype=bf)
    sel = np.zeros((128, C), dtype=bf)
    for g in range(NGRP):
        for c in range(C):
            sel[32 * g + c, c] = bf(1.0)

    # iteration-0 distributions (host softmax + premix)
    mx = unaries.max(axis=0, keepdims=True)
    e = np.exp(unaries - mx, dtype=np.float32)
    p0 = e / e.sum(axis=0, keepdims=True)
    p0 = p0.T.astype(bf).astype(np.float32).T       # device pl is bf16
    pb0 = (Mbl_s.astype(np.float32) @ p0).astype(f8)     # [C, N]
    ps0 = (Msp_s.astype(np.float32) @ p0).astype(f8)
    wt0 = psi.astype(np.float32) @ ps0.astype(np.float32).T   # [M, C] f32
    wt0_hi = wt0.astype(bf)
    wt0_lo = (wt0 - wt0_hi.astype(np.float32)).astype(bf)
    # [128, KCH, CP]: pb0T[p, k, c] = pb0[c, 128k+p], zero-padded
    pb0T = np.zeros((128, KCH, CP), dtype=f8)
    pb0T[:, :, 0:C] = pb0.T.reshape(KCH, 128, C).transpose(1, 0, 2)

    qT_init = np.ascontiguousarray(unaries.T).astype(np.float32)
    return (ffa, fla_full, psiT, psi, m2pb, m2ps, idn, sel,
            pb0T, wt0_hi, wt0_lo, qT_init)


def _make_in_maps(inputs):
    unaries = np.asarray(inputs["unaries"], dtype=np.float32)
    feat = np.asarray(inputs["feat"], dtype=np.float32)
    sw = np.asarray(inputs["spatial_weights"], dtype=np.float32)
    bw = np.asarray(inputs["bilateral_weights"], dtype=np.float32)
    compat = np.asarray(inputs["compatibility_matrix"], dtype=np.float32)

    (ffa, fla_full, psiT, psi, m2pb, m2ps, idn, sel,
     pb0T, wt0_hi, wt0_lo, qT_init) = _host_prep(unaries, feat, sw, bw, compat)
    in_maps = []
    for r in range(R):
        jsl = slice(r * NL, (r + 1) * NL)
        ksl = slice(r * JCH, (r + 1) * JCH)
        in_maps.append({
            "ffa": ffa,
            "fla": np.ascontiguousarray(fla_full[:, jsl]),
            "psiTl": np.ascontiguousarray(psiT[:, ksl, :]),
            "psil": np.ascontiguousarray(psi[:, jsl]),
            "pb_init": pb0T,
            "wt0_hi": wt0_hi,
            "wt0_lo": wt0_lo,
            "unT_loc": np.ascontiguousarray(
                qT_init[jsl].reshape(JCH, 128, C).transpose(1, 0, 2)
            ),
            "m2pb": m2pb,
            "m2ps": m2ps,
            "idn": idn,
            "sel": sel,
        })
    return in_maps


def kernel(unaries, feat, spatial_weights, bilateral_weights, compatibility_matrix):
    from concourse.bass_utils import run_bass_kernel_spmd

    in_maps = _make_in_maps({
        "unaries": unaries,
        "feat": feat,
        "spatial_weights": spatial_weights,
        "bilateral_weights": bilateral_weights,
        "compatibility_matrix": compatibility_matrix,
    })
    nc = _get_program()
    res = run_bass_kernel_spmd(nc, in_maps, core_ids=list(range(R)))

    q = np.empty((C, N), dtype=np.float32)
    for r in range(R):
        out = res.results[r]["qT_out"]          # [128, JCH, C]
        q[:, r * NL:(r + 1) * NL] = out.transpose(2, 1, 0).reshape(C, NL)
    return q
